# revision 2
# baseline (speedup 1.0000x reference)
"""Multi-head attention (B=4, S=2048, D=1024, H=16) on 8 TRN2 NeuronCores, v3.

Sharding: core c <- (batch b = c // 2, head-group g = c % 2); head-group =
8 heads = 512 projection dims. Per core:

    QT = (q[b] @ Wq_g)^T   [512, S]  (bf16 at 32x scale, head-major)
    KT = (k[b] @ Wk_g)^T   [512, S]
    V  =  v[b] @ Wv_g      [S, 8*65] (64 data cols at 32x + 32.0 ones col)
    attention per head pair, q-tiles of 512:
        scoresT = K_h Q_h^T -> exp (ScalarE, scale SCALE/1024 folds the 32x
        W pre-scales) -> P^T bf16
        AV with P^T chunks [128k,128q] stationary and V [128k,65] moving;
        col 64 = 32*denominator. normalize via DVE reciprocal -> bf16
        PE transpose -> attn_out^T
    outT_partial = Wo_g^T @ attn_outT  [1024, S]

v3 over v2: the Q/K/V projections run as fp8e4 DoubleRow matmuls. Host
splits x and 32*W into (hi, lo) e4m3 planes (hi+lo recovers ~2^-8 relative
precision, better than bf16); each 512-col projection block is 12 DoubleRow
instructions (3 terms x 4 k-chunk-pairs, dropping the lo*lo term) at 0.5
cycles/col, i.e. 6/8 of the bf16 column cost. Scores/AV stay bf16 (their
contraction geometry gives fp8 no win at equal accuracy). The ScalarE exp
stream (256 tiles x ~1.04us) is the pacer; projections/output fill PE slack.

Host: out[b] = (outT_{b,0} + outT_{b,1})^T + bo + bv @ Wo.
"""

import numpy as np
import ml_dtypes

B, S, D, H = 4, 2048, 1024, 16
HD = 64
G = D // 2          # per-core head-group width = 512
NH = G // HD        # heads per core = 8
PAIRS = NH // 2
SCALE = 1.0 / np.sqrt(HD)
WSC = 32.0          # host pre-scale on Wq/Wk/Wv (power of 2, exact)

_CACHE = {}


def _split_multiwaits(nc, cap=1):
    """The walrus build in this container rejects instructions carrying more
    than `cap` sem waits (Tile's tail drain has 3). Move extra waits onto
    no-op instructions inserted just before, on the same engine."""
    import concourse.mybir as mybir

    n = 0
    for func in nc.m.functions:
        for blk in func.blocks:
            insts = list(blk.instructions)
            new_insts = []
            changed = False
            for inst in insts:
                si = inst.sync_info
                if si is not None and si.on_wait and len(si.on_wait) > cap:
                    waits = list(si.on_wait)
                    extra, keep = waits[:-cap], waits[-cap:]
                    for j, w in enumerate(extra):
                        nop = mybir.InstNoOp(
                            name=f"{inst.name}-wsplit{j}",
                            sync_info=mybir.SyncInfo(on_wait=[w], on_update=[]),
                            engine=inst.engine,
                            bass_nofuse=True,
                        )
                        new_insts.append(nop)
                        n += 1
                    inst.sync_info = mybir.SyncInfo(
                        on_wait=keep, on_update=list(si.on_update)
                    )
                    changed = True
                new_insts.append(inst)
            if changed:
                blk.instructions = new_insts
    return n


def _strip_ldweights(nc):
    """Drop Tile's separate Ldweights instructions; walrus handles
    self-loading matmuls. LDWs that carry sems become NoOps."""
    import concourse.mybir as mybir

    n = 0
    for func in nc.m.functions:
        for blk in func.blocks:
            insts = list(blk.instructions)
            new_insts = []
            changed = False
            for inst in insts:
                if type(inst).__name__ == "InstLdweights":
                    si = inst.sync_info
                    if si is not None and (si.on_wait or si.on_update):
                        new_insts.append(mybir.InstNoOp(
                            name=inst.name,
                            sync_info=mybir.SyncInfo(
                                on_wait=list(si.on_wait),
                                on_update=list(si.on_update)),
                            engine=inst.engine,
                            bass_nofuse=True,
                        ))
                    n += 1
                    changed = True
                    continue
                if type(inst).__name__ == "InstMatmult":
                    inst.ldweights = True
                new_insts.append(inst)
            if changed:
                blk.instructions = new_insts
    return n


def build_mha_nc(dbg=None):
    import concourse.bass as bass
    import concourse.mybir as mybir
    import concourse.tile as tile

    dt = mybir.dt
    f32 = dt.float32
    bf16 = dt.bfloat16
    fp8 = dt.float8e4
    Exp = mybir.ActivationFunctionType.Exp
    DR = mybir.MatmulPerfMode.DoubleRow

    s, d, g = S, D, G
    mch = g // 128        # head-pair chunks = 4
    kch = d // 128        # contraction chunks over D = 8
    sch = s // 128        # S chunks (k-chunks in attention) = 16
    qw = 512
    nqw = s // qw         # attention q-tiles per pair = 4
    sn = s // 512         # 512-wide blocks over S = 4

    nc = bass.Bass("TRN2", target_bir_lowering=False, debug=False)

    qTh = nc.declare_dram_parameter("qTh", [d, s], fp8, isOutput=False)
    qTl = nc.declare_dram_parameter("qTl", [d, s], fp8, isOutput=False)
    kTh = nc.declare_dram_parameter("kTh", [d, s], fp8, isOutput=False)
    kTl = nc.declare_dram_parameter("kTl", [d, s], fp8, isOutput=False)
    vTh = nc.declare_dram_parameter("vTh", [d, s], fp8, isOutput=False)
    vTl = nc.declare_dram_parameter("vTl", [d, s], fp8, isOutput=False)
    Wqh = nc.declare_dram_parameter("Wqh", [d, g], fp8, isOutput=False)
    Wql = nc.declare_dram_parameter("Wql", [d, g], fp8, isOutput=False)
    Wkh = nc.declare_dram_parameter("Wkh", [d, g], fp8, isOutput=False)
    Wkl = nc.declare_dram_parameter("Wkl", [d, g], fp8, isOutput=False)
    Wvh = nc.declare_dram_parameter("Wvh", [d, g], fp8, isOutput=False)
    Wvl = nc.declare_dram_parameter("Wvl", [d, g], fp8, isOutput=False)
    Wo = nc.declare_dram_parameter("Wo", [g, d], bf16, isOutput=False)
    bq = nc.declare_dram_parameter("bq", [128, mch], f32, isOutput=False)
    bk = nc.declare_dram_parameter("bk", [128, mch], f32, isOutput=False)
    ident = nc.declare_dram_parameter("ident", [128, 128], bf16,
                                      isOutput=False)
    outT = nc.declare_dram_parameter("outT", [d, s], bf16, isOutput=True)

    with tile.TileContext(nc) as tc:
        with (
            tc.tile_pool(name="const", bufs=1) as const,
            tc.tile_pool(name="acts", bufs=1) as acts,
            tc.tile_pool(name="inT", bufs=1) as inT_pool,
            tc.tile_pool(name="pTp", bufs=2) as pTp,
            tc.tile_pool(name="nrm", bufs=2) as nrm,
            tc.tile_pool(name="outsb", bufs=5) as outp,
            tc.tile_pool(name="ps", bufs=2,
                         space=bass.MemorySpace.PSUM) as ps,
        ):
            # ---- constants ----
            Wqh_sb = const.tile([128, kch, g], fp8)
            Wql_sb = const.tile([128, kch, g], fp8)
            Wkh_sb = const.tile([128, kch, g], fp8)
            Wkl_sb = const.tile([128, kch, g], fp8)
            Wvh_sb = const.tile([128, kch, g], fp8)
            Wvl_sb = const.tile([128, kch, g], fp8)
            Wo_sb = const.tile([128, mch, d], bf16)
            bq_sb = const.tile([128, mch], f32)
            bk_sb = const.tile([128, mch], f32)
            ident_sb = const.tile([128, 128], bf16)
            nc.sync.dma_start(bq_sb[:], bq[:])
            nc.sync.dma_start(bk_sb[:], bk[:])
            ones_sb = const.tile([128, 512], bf16)
            nc.vector.memset(ones_sb[:], 1.0)

            # ---- resident activations ----
            QT_sb = acts.tile([128, mch, s], bf16)   # Q^T head-major, 32x
            KT_sb = acts.tile([128, mch, s], bf16)
            # V natural [S, nh*65]: 64 data cols (32x) + a 32.0 ones column
            # per head (65th moving col of AV = 32 * softmax denominator).
            V_sb = acts.tile([128, sch, NH * 65], bf16)
            AOT_sb = acts.tile([128, mch, s], bf16)  # attn_out^T (scale 1)
            nc.vector.memset(
                V_sb.rearrange("p s (h c) -> p s h c", c=65)[:, :, :, 64:65],
                float(WSC))

            # fp8 hi/lo input planes, [128, kch, s]
            vTh_t = inT_pool.tile([128, kch, s], fp8, name="vTh_t")
            vTl_t = inT_pool.tile([128, kch, s], fp8, name="vTl_t")
            kTh_t = inT_pool.tile([128, kch, s], fp8, name="kTh_t")
            kTl_t = inT_pool.tile([128, kch, s], fp8, name="kTl_t")
            qTh_t = inT_pool.tile([128, kch, s], fp8, name="qTh_t")
            qTl_t = inT_pool.tile([128, kch, s], fp8, name="qTl_t")

            def dma_block(src, t, c0, c1):
                # one instruction moves the column block of ALL 8 d-chunks
                nc.sync.dma_start(
                    t[:, :, c0:c1],
                    src.rearrange("(c p) n -> p c n", p=128)[:, :, c0:c1])

            def dma_w(src, t, m0, m1):
                # W column block (m-chunk granularity) over all 8 d-chunks
                nc.sync.dma_start(
                    t[:, :, m0:m1],
                    src.rearrange("(c p) n -> p c n", p=128)[:, :, m0:m1])

            # DMA order = consumption order; the sync (HWDGE) queue issues
            # in emission order and DMA_ENGINES serializes at ~360B/ns, so
            # earliest-deadline-first. Unit 0 is DMA-paced end to end: its
            # gate set is kT (4MB) + vT (4MB) + qT tile0 (1MB) + Wk/Wq m0
            # (0.5MB) + Wv cols 0:256 (0.5MB) ~= 10MB ~= 28us.
            dma_w(Wkh, Wkh_sb, 0, 128)
            dma_block(kTh, kTh_t, 0, 128)
            dma_w(Wqh, Wqh_sb, 0, 128)
            dma_block(qTh, qTh_t, 0, 512)
            dma_w(Wkl, Wkl_sb, 0, 128)
            dma_block(kTl, kTl_t, 0, 128)
            dma_w(Wql, Wql_sb, 0, 128)
            dma_block(qTl, qTl_t, 0, 512)
            # first exp possible here (~5us); unit-0 kc stream follows
            dma_block(kTh, kTh_t, 128, 512)
            dma_block(kTl, kTl_t, 128, 512)
            dma_w(Wvh, Wvh_sb, 0, 256)
            dma_w(Wvl, Wvl_sb, 0, 256)
            dma_block(vTh, vTh_t, 0, 512)
            dma_block(vTl, vTl_t, 0, 512)
            dma_block(kTh, kTh_t, 512, 1024)
            dma_block(kTl, kTl_t, 512, 1024)
            dma_block(vTh, vTh_t, 512, 1024)
            dma_block(vTl, vTl_t, 512, 1024)
            dma_block(kTh, kTh_t, 1024, 2048)
            dma_block(kTl, kTl_t, 1024, 2048)
            dma_block(vTh, vTh_t, 1024, 2048)
            dma_block(vTl, vTl_t, 1024, 2048)
            # unit 1+ needs
            dma_block(qTh, qTh_t, 512, 1024)
            dma_block(qTl, qTl_t, 512, 1024)
            nc.sync.dma_start(ident_sb[:], ident[:])
            dma_w(Wvh, Wvh_sb, 256, 512)
            dma_w(Wvl, Wvl_sb, 256, 512)
            dma_block(qTh, qTh_t, 1024, 2048)
            dma_block(qTl, qTl_t, 1024, 2048)
            dma_w(Wkh, Wkh_sb, 128, 512)
            dma_w(Wkl, Wkl_sb, 128, 512)
            dma_w(Wqh, Wqh_sb, 128, 512)
            dma_w(Wql, Wql_sb, 128, 512)
            nc.sync.dma_start(
                Wo_sb[:], Wo.rearrange("(c p) n -> p c n", p=128))

            # ---------------- emission helpers ----------------
            CPAIRS = [(0, 2), (2, 4), (4, 6), (6, 8)]

            def dr3(out_ap, Wh, Wl, xh, xl, wslice, xslice, cost):
                """Yield the 12 DoubleRow matmuls of a 3-term fp8 block:
                Wh.xh + Wl.xh + Wh.xl over 4 k-chunk pairs, one psum group."""
                first = True
                terms = ([(Wh, xh, c) for c in CPAIRS]
                         + [(Wl, xh, c) for c in CPAIRS]
                         + [(Wh, xl, c) for c in CPAIRS])
                n = len(terms)
                for i, (Wt, xt, (c0, c1)) in enumerate(terms):
                    nc.tensor.matmul(
                        out_ap,
                        Wt[:, c0:c1, wslice],
                        xt[:, c0:c1, xslice],
                        start=(i == 0), stop=(i == n - 1),
                        perf_mode=DR,
                    )
                    yield cost

            def v_proj_chunk(sc, glo, ghi):
                """V projection for s-chunk sc, g columns [glo, ghi)."""
                w = ghi - glo
                vp = ps.tile([128, 512], f32, tag="px", name="vp")
                scs = slice(sc * 128, (sc + 1) * 128)
                yield from dr3(vp[:, 0:w], vTh_t, vTl_t, Wvh_sb, Wvl_sb,
                               scs, slice(glo, ghi), 0.21 * w)
                # bv is folded into bo on the host.
                h0 = glo // HD
                nc.vector.tensor_copy(
                    V_sb[:, sc].rearrange(
                        "p (h c) -> p h c", c=65)[:, h0:h0 + w // HD, 0:64],
                    vp[:, 0:w].rearrange("p (h c) -> p h c", c=64))
                yield 0.42 * w

            def qk_proj_chunk(Wh, Wl, xh, xl, b_sb, dst, m, n0, n1):
                """Q/K projection chunk: m-chunk m (pair), cols [n0, n1)."""
                w = n1 - n0
                pp = ps.tile([128, 512], f32, tag="px", name="pp")
                yield from dr3(pp[:, 0:w], Wh, Wl, xh, xl,
                               slice(m * 128, (m + 1) * 128),
                               slice(n0, n1), 0.21 * w)
                nc.vector.tensor_scalar_add(
                    dst[:, m, n0:n1], pp[:, 0:w], b_sb[:, m:m + 1])
                yield 20.0

            def outproj_chunk(mo, n0, tag, on_act=False):
                """Output projection chunk outT[mo*128:, n0*512:]."""
                op = ps.tile([128, 1024] if tag == "sc" else [128, 512],
                             f32, tag=tag, name="op")
                for kk in range(mch):
                    nc.tensor.matmul(
                        op[:, 0:512],
                        Wo_sb[:, kk, mo * 128:(mo + 1) * 128],
                        AOT_sb[:, kk, n0 * 512:(n0 + 1) * 512],
                        start=(kk == 0), stop=(kk == mch - 1))
                    yield 213.0
                ot = outp.tile([128, 512], bf16, tag="ot", name="ot")
                if on_act:
                    nc.scalar.activation(
                        ot[:], op[:, 0:512],
                        mybir.ActivationFunctionType.Copy)
                else:
                    nc.vector.tensor_copy(ot[:], op[:, 0:512])
                nc.sync.dma_start(
                    outT[mo * 128:(mo + 1) * 128,
                         n0 * 512:(n0 + 1) * 512], ot[:])
                yield 20.0

            # ---------------- prologue ----------------
            # PE p-state warmup: dependency-free matmuls burn the 3us ramp
            # while the first input DMAs stream.
            for _ in range(10):
                wt = ps.tile([128, 512], f32, tag="px", name="wt")
                nc.tensor.matmul(wt[:], ones_sb[:, 0:128], ones_sb[:],
                                 start=True, stop=True)
            # Just enough to start attention: K(pair 0) cols 0:128,
            # Q(pair 0, qtile 0). Everything else streams in as fillers.
            for _ in qk_proj_chunk(Wkh_sb, Wkl_sb, kTh_t, kTl_t, bk_sb,
                                   KT_sb, 0, 0, 128):
                pass
            for _ in qk_proj_chunk(Wqh_sb, Wql_sb, qTh_t, qTl_t, bq_sb,
                                   QT_sb, 0, 0, 512):
                pass

            # ---------------- filler schedule ----------------
            fillers = []
            # K(pair 0) cols 128:512 feeds scores kc 1..3
            fillers.append((0, qk_proj_chunk(Wkh_sb, Wkl_sb, kTh_t, kTl_t,
                                             bk_sb, KT_sb, 0, 128, 512)))
            for sc in range(sch):
                # V pairs 0-1 s-chunk sc feeds the (lag-1) AV batch of
                # kc == sc, emitted at slot sc+1.
                fillers.append((max(0 * 16 + sc - 1, 1),
                                v_proj_chunk(sc, 0, 256)))
            for n0 in range(1, sn):
                # KT(pair 0) block n0 feeds scores at kc == 4*n0
                fillers.append((0 * 16 + 4 * n0 - 2,
                                qk_proj_chunk(Wkh_sb, Wkl_sb, kTh_t, kTl_t,
                                              bk_sb, KT_sb, 0,
                                              n0 * 512, (n0 + 1) * 512)))
            for sc in range(sch):
                # V pairs 2-3: hard deadline is (u8, kc=sc); spread earlier.
                fillers.append((min(32 + 6 * sc, 8 * 16 + sc - 2),
                                v_proj_chunk(sc, 256, 512)))
            units = [(0, 0), (0, 1), (0, 2), (0, 3),
                     (1, 0), (1, 1), (1, 2), (1, 3),
                     (2, 0), (3, 0), (2, 1), (3, 1),
                     (2, 2), (3, 2), (2, 3), (3, 3)]
            uidx = {pt: i for i, pt in enumerate(units)}
            first_u = {0: 0, 1: 4, 2: 8, 3: 9}
            for p in (1, 2, 3):
                for n0 in range(sn):
                    fillers.append((first_u[p] * 16 + 4 * n0 - 2,
                                    qk_proj_chunk(Wkh_sb, Wkl_sb, kTh_t,
                                                  kTl_t, bk_sb, KT_sb, p,
                                                  n0 * 512, (n0 + 1) * 512)))
            for pr in range(PAIRS):
                for t in range(nqw):
                    if (pr, t) == (0, 0):
                        continue
                    fillers.append((uidx[(pr, t)] * 16 - 4,
                                    qk_proj_chunk(Wqh_sb, Wql_sb, qTh_t,
                                                  qTl_t, bq_sb, QT_sb, pr,
                                                  t * 512, (t + 1) * 512)))
            fillers.sort(key=lambda x: x[0])
            from collections import deque
            fq = deque(fillers)
            oq = deque()

            # SINGLE-FLIGHT px discipline (see v2): at most one open chunk
            # generator may be suspended at a time.
            cur = {"gen": None, "dl": None}

            def finish_cur():
                if cur["gen"] is not None:
                    for c in cur["gen"]:
                        clk["vpe"] += c or 213.0
                    cur["gen"] = None

            def pull(slot, budget):
                spent = 0
                while True:
                    if clk["on"]:
                        if clk["vpe"] + 120.0 > clk["vact"] - 1100.0:
                            return
                    elif spent >= budget:
                        return
                    if cur["gen"] is None:
                        if fq and fq[0][0] <= slot + 32:
                            cur["dl"], cur["gen"] = fq.popleft()
                        elif oq:
                            mo, n0 = oq.popleft()
                            cur["gen"] = outproj_chunk(mo, n0, "px")
                            cur["dl"] = None
                        else:
                            return
                    try:
                        clk["vpe"] += next(cur["gen"]) or 213.0
                        spent += 1
                    except StopIteration:
                        cur["gen"] = None

            def drain(slot):
                while True:
                    if (cur["gen"] is not None and cur["dl"] is not None
                            and cur["dl"] <= slot):
                        finish_cur()
                        continue
                    if fq and fq[0][0] <= slot:
                        finish_cur()
                        cur["dl"], cur["gen"] = fq.popleft()
                        finish_cur()
                        continue
                    break

            # ---------------- attention ----------------
            deferred = []   # (earliest_slot, pe_cost_ns, closure)
            clk = {"vpe": 0.0, "vact": 0.0, "on": False}

            def emit_deferred(slot):
                rest = []
                for es, cost, fn in deferred:
                    if es <= slot:
                        fn()
                        clk["vpe"] += cost
                    else:
                        rest.append((es, cost, fn))
                deferred[:] = rest

            done_t = [0] * nqw
            for u, (pr, t) in enumerate(units):
                if True:
                    hA, hB = 2 * pr, 2 * pr + 1
                    gl = slice(t * qw, (t + 1) * qw)
                    avA = ps.tile([128, 4, 65], f32, tag="av", name="avA")
                    avB = ps.tile([128, 4, 65], f32, tag="av", name="avB")
                    for kc in range(sch):
                        drain(u * 16 + kc)
                        kcs = slice(kc * 128, kc * 128 + 128)
                        scAB = ps.tile([128, 1024], f32, tag="sc",
                                       name="scAB")
                        nc.tensor.matmul(
                            scAB[:, 0:qw], KT_sb[0:64, pr, kcs],
                            QT_sb[0:64, pr, gl], start=True, stop=True)
                        nc.tensor.matmul(
                            scAB[:, qw:2 * qw], KT_sb[64:128, pr, kcs],
                            QT_sb[64:128, pr, gl], start=True, stop=True)
                        pT = pTp.tile([128, 1024], bf16, tag="pT", name="pT")
                        nc.scalar.activation(pT[:], scAB[:], Exp,
                                             scale=float(SCALE / (WSC * WSC)))

                        def av_batch(kc=kc, pT=pT, avA=avA, avB=avB,
                                     hA=hA, hB=hB):
                            for hoff, h, av in ((0, hA, avA), (qw, hB, avB)):
                                for qc in range(4):
                                    nc.tensor.matmul(
                                        av[:, qc, :],
                                        pT[:, hoff + qc * 128:
                                           hoff + (qc + 1) * 128],
                                        V_sb[:, kc, h * 65:h * 65 + 65],
                                        start=(kc == 0 and qc == 0),
                                        stop=(kc == sch - 1 and qc == 3))
                        slot = u * 16 + kc
                        clk["vpe"] += 427.0
                        clk["vact"] = max(clk["vact"],
                                          clk["vpe"] + 100.0) + 1038.0
                        deferred.append((slot + 1, 217.0, av_batch))
                        emit_deferred(slot)
                        pull(slot, 2 if u < 12 else 4)

                    state = {}

                    def make_norm(avA=avA, avB=avB, state=state,
                                  final=(u == len(units) - 1)):
                        def norm():
                            recA = nrm.tile([128, 4], f32, tag="recA",
                                            name="recA")
                            recB = nrm.tile([128, 4], f32, tag="recB",
                                            name="recB")
                            avn = nrm.tile([128, 4, 128], bf16, tag="avn",
                                           name="avn")
                            if final:
                                srcA, srcB = avA, avB
                            else:
                                srcA = nrm.tile([128, 4, 65], f32, tag="cpA",
                                                name="cpA", bufs=1)
                                srcB = nrm.tile([128, 4, 65], f32, tag="cpB",
                                                name="cpB", bufs=1)
                                nc.vector.tensor_copy(srcA[:], avA[:])
                                nc.vector.tensor_copy(srcB[:], avB[:])
                            nc.vector.reciprocal(recA[:], srcA[:, :, 64])
                            for qc in range(4):
                                nc.vector.tensor_scalar_mul(
                                    avn[:, qc, 0:64], srcA[:, qc, 0:64],
                                    recA[:, qc:qc + 1])
                            nc.vector.reciprocal(recB[:], srcB[:, :, 64])
                            for qc in range(4):
                                nc.vector.tensor_scalar_mul(
                                    avn[:, qc, 64:128], srcB[:, qc, 0:64],
                                    recB[:, qc:qc + 1])
                            state["avn"] = avn
                        return norm

                    def make_transp(pr=pr, t=t, state=state):
                        def transp():
                            finish_cur()
                            avn = state["avn"]
                            aot = ps.tile([128, 512], bf16, tag="px",
                                          name="aot")
                            for qc in range(4):
                                nc.tensor.matmul(
                                    aot[:, qc * 128:(qc + 1) * 128],
                                    avn[:, qc, :], ident_sb[:],
                                    is_transpose=True,
                                    start=(qc == 0), stop=(qc == 3))
                            nc.vector.tensor_copy(
                                AOT_sb[:, pr, t * qw:(t + 1) * qw], aot[:])
                            done_t[t] += 1
                            if done_t[t] == PAIRS:
                                for mo in range(d // 128):
                                    oq.append((mo, t))
                        return transp

                    last = u * 16 + 15
                    deferred.append((last + 1, 0.0, make_norm()))
                    deferred.append((last + 3, 220.0, make_transp()))
                    if u == 0:
                        clk["on"] = True
                        clk["vpe"] = clk["vact"] - 400.0

            # flush the deferral queue (last unit's av batch + tail)
            for es, cost, fn in deferred:
                fn()
            deferred[:] = []

            # ---------------- epilogue ----------------
            drain(10 ** 6)
            finish_cur()
            ntag = 0
            while oq:
                mo, n0 = oq.popleft()
                for _ in outproj_chunk(mo, n0, ("px", "sc")[ntag % 2]):
                    pass
                ntag += 1

    if not dbg:
        _strip_ldweights(nc)
        _split_multiwaits(nc, cap=1)
    return nc


def _get_nc():
    if "nc" not in _CACHE:
        _CACHE["nc"] = build_mha_nc()
    return _CACHE["nc"]


def _split8(x):
    f8 = ml_dtypes.float8_e4m3
    hi = x.astype(f8)
    lo = (x - hi.astype(np.float32)).astype(f8)
    return hi, lo


def make_in_maps(q, k, v, Wq, bq, Wk, bk, Wv, bv, Wo, bo, **_ignored):
    """Shard + lay out the full inputs for the 8 cores."""
    bf = ml_dtypes.bfloat16
    q = np.asarray(q, np.float32)
    k = np.asarray(k, np.float32)
    v = np.asarray(v, np.float32)
    Wq = np.asarray(Wq, np.float32)
    Wk = np.asarray(Wk, np.float32)
    Wv = np.asarray(Wv, np.float32)
    Wo = np.asarray(Wo, np.float32)
    bq = np.asarray(bq, np.float32)
    bk = np.asarray(bk, np.float32)
    eye = np.eye(128, dtype=bf)

    xplanes = {}
    for name, x in (("q", q), ("k", k), ("v", v)):
        for b in range(B):
            xplanes[(name, b)] = _split8(np.ascontiguousarray(x[b].T))

    in_maps = []
    for c in range(8):
        b, gi = divmod(c, 2)
        gs = slice(gi * G, (gi + 1) * G)
        qh, ql = xplanes[("q", b)]
        kh, kl = xplanes[("k", b)]
        vh, vl = xplanes[("v", b)]
        Wqh, Wql = _split8(np.ascontiguousarray(Wq[:, gs]) * WSC)
        Wkh, Wkl = _split8(np.ascontiguousarray(Wk[:, gs]) * WSC)
        Wvh, Wvl = _split8(np.ascontiguousarray(Wv[:, gs]) * WSC)
        in_maps.append({
            "qTh": qh, "qTl": ql,
            "kTh": kh, "kTl": kl,
            "vTh": vh, "vTl": vl,
            "Wqh": Wqh, "Wql": Wql,
            "Wkh": Wkh, "Wkl": Wkl,
            "Wvh": Wvh, "Wvl": Wvl,
            "Wo": np.ascontiguousarray(Wo[gs, :]).astype(bf),
            "bq": np.ascontiguousarray(
                (bq[gs] * WSC).reshape(G // 128, 128).T),
            "bk": np.ascontiguousarray(
                (bk[gs] * WSC).reshape(G // 128, 128).T),
            "ident": eye,
        })
    return in_maps


def run(in_maps, trace=False, trace_kwargs=None):
    from concourse.bass_utils import run_bass_kernel_spmd

    nc = _get_nc()
    kw = {}
    if trace:
        kw["trace"] = True
        kw.update(trace_kwargs or {})
    return run_bass_kernel_spmd(nc, in_maps, core_ids=list(range(8)), **kw)


def kernel(q, k, v, Wq, bq, Wk, bk, Wv, bv, Wo, bo, **_ignored):
    in_maps = make_in_maps(q, k, v, Wq, bq, Wk, bk, Wv, bv, Wo, bo)
    res = run(in_maps)
    bo = np.asarray(bo, np.float32)
    bv64 = np.asarray(bv, np.float64)
    Wo64 = np.asarray(Wo, np.float64)
    # bv rides through the softmax weighting unchanged: out += bv @ Wo
    bo_eff = (bo.astype(np.float64) + bv64 @ Wo64).astype(np.float32)
    out = np.empty((B, S, D), np.float32)
    for b in range(B):
        acc = (res.results[2 * b]["outT"].astype(np.float32)
               + res.results[2 * b + 1]["outT"].astype(np.float32))
        out[b] = acc.T + bo_eff[None, :]
    return out


# revision 34
# speedup vs baseline: 2.0447x; 2.0447x over previous
"""Multi-head attention (B=4, S=2048, D=1024, H=16) on 8 TRN2 NeuronCores, v3.

Sharding: core c <- (batch b = c // 2, head-group g = c % 2); head-group =
8 heads = 512 projection dims. Per core:

    QT = (q[b] @ Wq_g)^T   [512, S]  (bf16 at 32x scale, head-major)
    KT = (k[b] @ Wk_g)^T   [512, S]
    V  =  v[b] @ Wv_g      [S, 8*65] (64 data cols at 32x + 32.0 ones col)
    attention per head pair, q-tiles of 512:
        scoresT = K_h Q_h^T -> exp (ScalarE, scale SCALE/1024 folds the 32x
        W pre-scales) -> P^T bf16
        AV with P^T chunks [128k,128q] stationary and V [128k,65] moving;
        col 64 = 32*denominator. normalize via DVE reciprocal -> bf16
        PE transpose -> attn_out^T
    outT_partial = Wo_g^T @ attn_outT  [1024, S]

v3 over v2: the Q/K/V projections run as fp8e4 DoubleRow matmuls. Host
splits x and 32*W into (hi, lo) e4m3 planes (hi+lo recovers ~2^-8 relative
precision, better than bf16); each 512-col projection block is 12 DoubleRow
instructions (3 terms x 4 k-chunk-pairs, dropping the lo*lo term) at 0.5
cycles/col, i.e. 6/8 of the bf16 column cost. Scores/AV stay bf16 (their
contraction geometry gives fp8 no win at equal accuracy). The ScalarE exp
stream (256 tiles x ~1.04us) is the pacer; projections/output fill PE slack.

Host: out[b] = (outT_{b,0} + outT_{b,1})^T + bo + bv @ Wo.
"""

import numpy as np
import ml_dtypes

B, S, D, H = 4, 2048, 1024, 16
HD = 64
G = D // 2          # per-core head-group width = 512
NH = G // HD        # heads per core = 8
PAIRS = NH // 2
SCALE = 1.0 / np.sqrt(HD)
WSC = 32.0          # host pre-scale on Wq/Wk/Wv (power of 2, exact)

_CACHE = {}

# schedule knobs (swept offline with TimelineSim)
CFG = {"tail_budget": 8, "wm1_pos": "late", "act_copies": True,
       "transp_off": 6,
       "warmups": 11,
       "era_budget": 10,
       "pull_guard": 1100.0, "clamp": 2200.0}


def _split_multiwaits(nc, cap=1):
    """The walrus build in this container rejects instructions carrying more
    than `cap` sem waits (Tile's tail drain has 3). Move extra waits onto
    no-op instructions inserted just before, on the same engine."""
    import concourse.mybir as mybir

    n = 0
    for func in nc.m.functions:
        for blk in func.blocks:
            insts = list(blk.instructions)
            new_insts = []
            changed = False
            for inst in insts:
                si = inst.sync_info
                if si is not None and si.on_wait and len(si.on_wait) > cap:
                    waits = list(si.on_wait)
                    extra, keep = waits[:-cap], waits[-cap:]
                    for j, w in enumerate(extra):
                        nop = mybir.InstNoOp(
                            name=f"{inst.name}-wsplit{j}",
                            sync_info=mybir.SyncInfo(on_wait=[w], on_update=[]),
                            engine=inst.engine,
                            bass_nofuse=True,
                        )
                        new_insts.append(nop)
                        n += 1
                    inst.sync_info = mybir.SyncInfo(
                        on_wait=keep, on_update=list(si.on_update)
                    )
                    changed = True
                new_insts.append(inst)
            if changed:
                blk.instructions = new_insts
    return n


def _strip_ldweights(nc):
    """Drop Tile's separate Ldweights instructions; walrus handles
    self-loading matmuls. LDWs that carry sems become NoOps."""
    import concourse.mybir as mybir

    n = 0
    for func in nc.m.functions:
        for blk in func.blocks:
            insts = list(blk.instructions)
            new_insts = []
            changed = False
            for inst in insts:
                if type(inst).__name__ == "InstLdweights":
                    si = inst.sync_info
                    if si is not None and (si.on_wait or si.on_update):
                        new_insts.append(mybir.InstNoOp(
                            name=inst.name,
                            sync_info=mybir.SyncInfo(
                                on_wait=list(si.on_wait),
                                on_update=list(si.on_update)),
                            engine=inst.engine,
                            bass_nofuse=True,
                        ))
                    n += 1
                    changed = True
                    continue
                if type(inst).__name__ == "InstMatmult":
                    inst.ldweights = True
                new_insts.append(inst)
            if changed:
                blk.instructions = new_insts
    return n


def build_mha_nc(dbg=None):
    import concourse.bass as bass
    import concourse.mybir as mybir
    import concourse.tile as tile

    dt = mybir.dt
    f32 = dt.float32
    bf16 = dt.bfloat16
    fp8 = dt.float8e4
    Exp = mybir.ActivationFunctionType.Exp
    DR = mybir.MatmulPerfMode.DoubleRow

    s, d, g = S, D, G
    mch = g // 128        # head-pair chunks = 4
    kch = d // 128        # contraction chunks over D = 8
    sch = s // 128        # S chunks (k-chunks in attention) = 16
    qw = 512
    nqw = s // qw         # attention q-tiles per pair = 4
    sn = s // 512         # 512-wide blocks over S = 4

    nc = bass.Bass("TRN2", target_bir_lowering=False, debug=False)

    qTh = nc.declare_dram_parameter("qTh", [d, s], fp8, isOutput=False)
    qTl = nc.declare_dram_parameter("qTl", [d, s], fp8, isOutput=False)
    kTh = nc.declare_dram_parameter("kTh", [d, s], fp8, isOutput=False)
    kTl = nc.declare_dram_parameter("kTl", [d, s], fp8, isOutput=False)
    vTh = nc.declare_dram_parameter("vTh", [d, s], fp8, isOutput=False)
    vTl = nc.declare_dram_parameter("vTl", [d, s], fp8, isOutput=False)
    # W planes host-packed as [128, mch, kch, 128] (1024B contiguous per
    # (p, m) so DMA runs at full rate) and flattened to [128, 4096].
    Wqh = nc.declare_dram_parameter("Wqh", [128, d * g // 128], fp8,
                                    isOutput=False)
    Wql = nc.declare_dram_parameter("Wql", [128, d * g // 128], fp8,
                                    isOutput=False)
    Wkh = nc.declare_dram_parameter("Wkh", [128, d * g // 128], fp8,
                                    isOutput=False)
    Wkl = nc.declare_dram_parameter("Wkl", [128, d * g // 128], fp8,
                                    isOutput=False)
    Wvh = nc.declare_dram_parameter("Wvh", [128, d * g // 128], fp8,
                                    isOutput=False)
    Wvl = nc.declare_dram_parameter("Wvl", [128, d * g // 128], fp8,
                                    isOutput=False)
    Wo = nc.declare_dram_parameter("Wo", [g, d], bf16, isOutput=False)
    bq = nc.declare_dram_parameter("bq", [128, mch], f32, isOutput=False)
    bk = nc.declare_dram_parameter("bk", [128, mch], f32, isOutput=False)
    ident = nc.declare_dram_parameter("ident", [128, 128], bf16,
                                      isOutput=False)
    outT = nc.declare_dram_parameter("outT", [d, s], bf16, isOutput=True)

    with tile.TileContext(nc) as tc:
        with (
            tc.tile_pool(name="const", bufs=1) as const,
            tc.tile_pool(name="acts", bufs=1) as acts,
            tc.tile_pool(name="inT", bufs=1) as inT_pool,
            tc.tile_pool(name="pTp", bufs=2) as pTp,
            tc.tile_pool(name="nrm", bufs=2) as nrm,
            tc.tile_pool(name="outsb", bufs=5) as outp,
            tc.tile_pool(name="ps", bufs=2,
                         space=bass.MemorySpace.PSUM) as ps,
        ):
            # ---- constants ----
            Wqh_sb = const.tile([128, mch, kch, 128], fp8)
            Wql_sb = const.tile([128, mch, kch, 128], fp8)
            Wkh_sb = const.tile([128, mch, kch, 128], fp8)
            Wkl_sb = const.tile([128, mch, kch, 128], fp8)
            Wvh_sb = const.tile([128, mch, kch, 128], fp8)
            Wvl_sb = const.tile([128, mch, kch, 128], fp8)
            Wo_sb = const.tile([128, mch, d], bf16)
            bq_sb = const.tile([128, mch], f32)
            bk_sb = const.tile([128, mch], f32)
            ident_sb = const.tile([128, 128], bf16)
            ones_sb = const.tile([128, 512], bf16)
            nc.vector.memset(ones_sb[:], 1.0)

            # ---- resident activations ----
            QT_sb = acts.tile([128, mch, s], bf16)   # Q^T head-major, 32x
            KT_sb = acts.tile([128, mch, s], bf16)
            # V natural [S, nh*65]: 64 data cols (32x) + a 32.0 ones column
            # per head (65th moving col of AV = 32 * softmax denominator).
            V_sb = acts.tile([128, sch, NH * 65], bf16)
            AOT_sb = acts.tile([128, mch, s], bf16)  # attn_out^T (scale 1)
            nc.vector.memset(
                V_sb.rearrange("p s (h c) -> p s h c", c=65)[:, :, :, 64:65],
                float(WSC))

            # fp8 hi/lo input planes, [128, kch, s]
            vTh_t = inT_pool.tile([128, kch, s], fp8, name="vTh_t")
            vTl_t = inT_pool.tile([128, kch, s], fp8, name="vTl_t")
            kTh_t = inT_pool.tile([128, kch, s], fp8, name="kTh_t")
            kTl_t = inT_pool.tile([128, kch, s], fp8, name="kTl_t")
            qTh_t = inT_pool.tile([128, kch, s], fp8, name="qTh_t")
            qTl_t = inT_pool.tile([128, kch, s], fp8, name="qTl_t")

            # DMA arrival tracking: DMA_ENGINES serializes at ~360B/ns, so
            # arrival(ns) ~= cum_bytes/0.36 + sem-prop. Fillers become
            # pull-eligible only once their inputs have landed; pulling
            # earlier would wall the in-order PE queue behind a DMA wait.
            arr = {"bytes": 0.0}

            def _track(key, nbytes):
                arr["bytes"] += nbytes
                arr[key] = 2400.0 + arr["bytes"] / 360.0 + 900.0

            def dma_block(src, t, c0, c1, key="x"):
                # one instruction moves the column block of ALL 8 d-chunks
                nc.sync.dma_start(
                    t[:, :, c0:c1],
                    src.rearrange("(c p) n -> p c n", p=128)[:, :, c0:c1])
                _track(key, (c1 - c0) * d)

            def dma_w(src, t, m0, m1, key="x"):
                # W m-chunk block, host-packed: contiguous 1024B runs
                nc.sync.dma_start(
                    t[:, m0:m1],
                    src.rearrange("p (m c j) -> p m c j",
                                  m=mch, c=kch)[:, m0:m1])
                _track(key, (m1 - m0) * 128 * d)

            # DMA order = consumption order; the sync (HWDGE) queue issues
            # in emission order and DMA_ENGINES serializes at ~360B/ns, so
            # earliest-deadline-first. Unit 0 is DMA-paced end to end: its
            # gate set is kT (4MB) + vT (4MB) + qT tile0 (1MB) + Wk/Wq m0
            # (0.5MB) + Wv cols 0:256 (0.5MB) ~= 10MB ~= 28us.
            # Both bias chains gate the first scores; order so qTl and kTl
            # land back to back. All blocks >=512B elements (full DMA rate).
            dma_w(Wqh, Wqh_sb, 0, 1, key="x")
            dma_block(qTh, qTh_t, 0, 512, key="x")
            dma_w(Wkh, Wkh_sb, 0, 1, key="x")
            dma_block(kTh, kTh_t, 0, 512, key="x")
            dma_w(Wql, Wql_sb, 0, 1, key="wq0")
            dma_block(qTl, qTl_t, 0, 512, key="q0")
            nc.sync.dma_start(bq_sb[:], bq[:])
            _track("bq", 128 * 8)
            nc.sync.dma_start(bk_sb[:], bk[:])
            _track("bk", 128 * 8)
            dma_w(Wkl, Wkl_sb, 0, 1, key="wk0")
            dma_block(kTl, kTl_t, 0, 512, key="k3")
            # first exp ~11us; unit 0/1 stream kc in DMA pace from here
            dma_block(vTh, vTh_t, 0, 512, key="x")
            dma_block(vTl, vTl_t, 0, 512, key="v3")
            dma_w(Wvh, Wvh_sb, 0, 1, key="x")
            dma_w(Wvl, Wvl_sb, 0, 1, key="wv0")
            dma_block(kTh, kTh_t, 512, 1024, key="x")
            dma_block(kTl, kTl_t, 512, 1024, key="k7")
            dma_block(vTh, vTh_t, 512, 1024, key="x")
            dma_block(vTl, vTl_t, 512, 1024, key="v7")
            dma_block(kTh, kTh_t, 1024, 1536, key="x")
            dma_block(kTl, kTl_t, 1024, 1536, key="k11")
            dma_block(vTh, vTh_t, 1024, 1536, key="x")
            dma_block(vTl, vTl_t, 1024, 1536, key="v11")
            dma_block(kTh, kTh_t, 1536, 2048, key="x")
            dma_block(kTl, kTl_t, 1536, 2048, key="k15")
            dma_block(vTh, vTh_t, 1536, 2048, key="x")
            dma_block(vTl, vTl_t, 1536, 2048, key="v15")
            # units 1-3 need only their qT tiles
            dma_block(qTh, qTh_t, 512, 1024, key="x")
            dma_block(qTl, qTl_t, 512, 1024, key="q1")
            dma_block(qTh, qTh_t, 1024, 1536, key="x")
            dma_block(qTl, qTl_t, 1024, 1536, key="q2")
            dma_block(qTh, qTh_t, 1536, 2048, key="x")
            dma_block(qTl, qTl_t, 1536, 2048, key="q3")
            nc.sync.dma_start(ident_sb[:], ident[:])
            arr["bytes"] += 128 * 256
            # pair 1-3 weights (first needed at u4/u8/u9)
            dma_w(Wqh, Wqh_sb, 1, 2, key="x")
            dma_w(Wql, Wql_sb, 1, 2, key="wq1")
            dma_w(Wkh, Wkh_sb, 1, 2, key="x")
            dma_w(Wkl, Wkl_sb, 1, 2, key="wk1")
            dma_w(Wvh, Wvh_sb, 1, 2, key="x")
            dma_w(Wvl, Wvl_sb, 1, 2, key="wv1")
            dma_w(Wqh, Wqh_sb, 2, 3, key="x")
            dma_w(Wql, Wql_sb, 2, 3, key="wq2")
            dma_w(Wkh, Wkh_sb, 2, 3, key="x")
            dma_w(Wkl, Wkl_sb, 2, 3, key="wk2")
            dma_w(Wvh, Wvh_sb, 2, 3, key="x")
            dma_w(Wvl, Wvl_sb, 2, 3, key="wv2")
            dma_w(Wqh, Wqh_sb, 3, 4, key="x")
            dma_w(Wql, Wql_sb, 3, 4, key="wq3")
            dma_w(Wkh, Wkh_sb, 3, 4, key="x")
            dma_w(Wkl, Wkl_sb, 3, 4, key="wk3")
            dma_w(Wvh, Wvh_sb, 3, 4, key="x")
            dma_w(Wvl, Wvl_sb, 3, 4, key="wv3")
            nc.sync.dma_start(
                Wo_sb[:], Wo.rearrange("(c p) n -> p c n", p=128))

            # arrival(ns) -> earliest pull slot (first exp ~ T0)
            T0 = 12300.0

            def aslot(*keys):
                t = max(arr[k] for k in keys)
                return int(max(0.0, (t - T0) / 1038.0)) + 1

            # ---------------- emission helpers ----------------
            CPAIRS = [(0, 2), (2, 4), (4, 6), (6, 8)]

            def v_proj_chunk(sc, glo, ghi):
                """V projection for s-chunk sc, g block [glo, glo+128)."""
                gb = glo // 128
                vp = ps.tile([128, 512], f32, tag="px", name="vp")
                scs = slice(sc * 128, (sc + 1) * 128)
                # stationary = vT planes, moving = packed Wv m-chunk
                terms = ([(vTh_t, Wvh_sb, c) for c in CPAIRS]
                         + [(vTl_t, Wvh_sb, c) for c in CPAIRS]
                         + [(vTh_t, Wvl_sb, c) for c in CPAIRS])
                for i, (xt, Wt, (c0, c1)) in enumerate(terms):
                    nc.tensor.matmul(
                        vp[:, 0:128],
                        xt[:, c0:c1, scs],
                        Wt[:, gb, c0:c1, :],
                        start=(i == 0), stop=(i == len(terms) - 1),
                        perf_mode=DR,
                    )
                    yield 27.0
                # bv is folded into bo on the host.
                h0 = glo // HD
                nc.vector.tensor_copy(
                    V_sb[:, sc].rearrange(
                        "p (h c) -> p h c", c=65)[:, h0:h0 + 2, 0:64],
                    vp[:, 0:128].rearrange("p (h c) -> p h c", c=64))
                yield 55.0

            def qk_proj_chunk(Wh, Wl, xh, xl, b_sb, dst, m, n0, n1):
                """Q/K projection chunk: m-chunk m (pair), cols [n0, n1)."""
                w = n1 - n0
                pp = ps.tile([128, 512], f32, tag="px", name="pp")
                terms = ([(Wh, xh, c) for c in CPAIRS]
                         + [(Wl, xh, c) for c in CPAIRS]
                         + [(Wh, xl, c) for c in CPAIRS])
                for i, (Wt, xt, (c0, c1)) in enumerate(terms):
                    nc.tensor.matmul(
                        pp[:, 0:w],
                        Wt[:, m, c0:c1, :],
                        xt[:, c0:c1, n0:n1],
                        start=(i == 0), stop=(i == len(terms) - 1),
                        perf_mode=DR,
                    )
                    yield 0.21 * w
                nc.vector.tensor_scalar_add(
                    dst[:, m, n0:n1], pp[:, 0:w], b_sb[:, m:m + 1])
                yield 20.0

            def outproj_chunk(mo, n0, tag, on_act=False):
                """Output projection chunk outT[mo*128:, n0*512:]."""
                op = ps.tile([128, 1024] if tag == "sc" else [128, 512],
                             f32, tag=tag, name="op")
                for kk in range(mch):
                    nc.tensor.matmul(
                        op[:, 0:512],
                        Wo_sb[:, kk, mo * 128:(mo + 1) * 128],
                        AOT_sb[:, kk, n0 * 512:(n0 + 1) * 512],
                        start=(kk == 0), stop=(kk == mch - 1))
                    yield 213.0
                ot = outp.tile([128, 512], bf16, tag="ot", name="ot")
                if on_act:
                    nc.scalar.activation(
                        ot[:], op[:, 0:512],
                        mybir.ActivationFunctionType.Copy)
                else:
                    nc.vector.tensor_copy(ot[:], op[:, 0:512])
                nc.sync.dma_start(
                    outT[mo * 128:(mo + 1) * 128,
                         n0 * 512:(n0 + 1) * 512], ot[:])
                yield 20.0

            # ---------------- prologue ----------------
            # PE p-state warmup: dependency-free matmuls burn the 3us ramp
            # while the first input DMAs stream.
            for _ in range(CFG["warmups"]):
                wt = ps.tile([128, 512], f32, tag="px", name="wt")
                nc.tensor.matmul(wt[:], ones_sb[:, 0:128], ones_sb[:],
                                 start=True, stop=True)
            # Just enough to start attention: Q(pair 0, qtile 0) and
            # K(pair 0) cols 0:512. Everything else streams in as fillers.
            for _ in qk_proj_chunk(Wqh_sb, Wql_sb, qTh_t, qTl_t, bq_sb,
                                   QT_sb, 0, 0, 512):
                pass
            for _ in qk_proj_chunk(Wkh_sb, Wkl_sb, kTh_t, kTl_t, bk_sb,
                                   KT_sb, 0, 0, 512):
                pass

            # ---------------- filler schedule ----------------
            # (deadline, earliest, gen): deadline = last slot before which
            # the chunk must be fully EMITTED (consumer correctness);
            # earliest = slot at which its DMA inputs have landed (pulling
            # before that would wall the in-order PE queue on a DMA wait).
            KKEY = {0: "k3", 1: "k7", 2: "k11", 3: "k15"}
            units = [(0, 0), (0, 1), (0, 2), (0, 3),
                     (1, 0), (1, 1), (1, 2), (1, 3),
                     (2, 0), (3, 0), (2, 1), (3, 1),
                     (2, 2), (3, 2), (2, 3), (3, 3)]
            uidx = {pt: i for i, pt in enumerate(units)}
            first_u = {0: 0, 1: 4, 2: 8, 3: 9}
            fillers = []
            for gb, (dl0, step) in enumerate(((0, 1), (30, 2),
                                              (64, 3), (80, 3))):
                fu = first_u[gb] * 16
                for sc in range(sch):
                    # V g-block gb = heads of pair gb; needed at the AV of
                    # (first unit of that pair, kc=sc). The AV itself is
                    # arrival-gated (av_es), so the deadline may be pushed
                    # to the data arrival to avoid premature force-drains.
                    es = aslot(f"wv{gb}", f"v{(sc // 4) * 4 + 3}")
                    dl = max(min(dl0 + step * sc, fu + sc - 1), 1, es)
                    fillers.append((dl, es,
                                    v_proj_chunk(sc, gb * 128, gb * 128 + 128)))
            for p in range(4):
                for n0 in range(sn):
                    if p == 0 and n0 == 0:
                        continue  # prologue
                    fillers.append((first_u[p] * 16 + 4 * n0 - 2,
                                    aslot(f"wk{p}", KKEY[n0]),
                                    qk_proj_chunk(Wkh_sb, Wkl_sb, kTh_t,
                                                  kTl_t, bk_sb, KT_sb, p,
                                                  n0 * 512, (n0 + 1) * 512)))
            for pr in range(PAIRS):
                for t in range(nqw):
                    if (pr, t) == (0, 0):
                        continue
                    fillers.append((uidx[(pr, t)] * 16,
                                    aslot(f"wq{pr}", f"q{t}"),
                                    qk_proj_chunk(Wqh_sb, Wql_sb, qTh_t,
                                                  qTl_t, bq_sb, QT_sb, pr,
                                                  t * 512, (t + 1) * 512)))
            fillers.sort(key=lambda x: x[0])
            fq = list(fillers)
            from collections import deque
            oq = deque()

            # SINGLE-FLIGHT px discipline (see v2): at most one open chunk
            # generator may be suspended at a time.
            cur = {"gen": None, "dl": None}

            def finish_cur():
                if cur["gen"] is not None:
                    for c in cur["gen"]:
                        clk["vpe"] += c or 213.0
                    cur["gen"] = None

            def pull(slot, budget):
                spent = 0
                while True:
                    if clk["on"]:
                        if clk["vpe"] + 120.0 > clk["vact"] - CFG["pull_guard"]:
                            return
                    elif spent >= budget:
                        return
                    if cur["gen"] is None:
                        pick = None
                        for i, (dl, es, _gen) in enumerate(fq):
                            if dl > slot + 32:
                                break
                            if es <= slot:
                                pick = i
                                break
                        if pick is not None:
                            cur["dl"], _, cur["gen"] = fq.pop(pick)
                        elif oq:
                            mo, n0 = oq.popleft()
                            cur["gen"] = outproj_chunk(mo, n0, "px")
                            cur["dl"] = None
                        else:
                            return
                    try:
                        clk["vpe"] += next(cur["gen"]) or 213.0
                        spent += 1
                    except StopIteration:
                        cur["gen"] = None

            def drain(slot):
                while True:
                    if (cur["gen"] is not None and cur["dl"] is not None
                            and cur["dl"] <= slot):
                        finish_cur()
                        continue
                    if fq and fq[0][0] <= slot:
                        finish_cur()
                        cur["dl"], _, cur["gen"] = fq.pop(0)
                        finish_cur()
                        continue
                    break

            # ---------------- attention ----------------
            deferred = []   # (earliest_slot, pe_cost_ns, closure)
            clk = {"vpe": 0.0, "vact": 0.0, "on": False}

            def emit_deferred(slot):
                rest = []
                for es, cost, fn in deferred:
                    if es <= slot:
                        fn()
                        clk["vpe"] += cost
                    else:
                        rest.append((es, cost, fn))
                deferred[:] = rest

            done_t = [0] * nqw
            for u, (pr, t) in enumerate(units):
                if True:
                    hA, hB = 2 * pr, 2 * pr + 1
                    gl = slice(t * qw, (t + 1) * qw)
                    avA = ps.tile([128, 4, 65], f32, tag="av", name="avA")
                    avB = ps.tile([128, 4, 65], f32, tag="av", name="avB")
                    for kc in range(sch):
                        drain(u * 16 + kc)
                        kcs = slice(kc * 128, kc * 128 + 128)
                        scAB = ps.tile([128, 1024], f32, tag="sc",
                                       name="scAB")
                        nc.tensor.matmul(
                            scAB[:, 0:qw], KT_sb[0:64, pr, kcs],
                            QT_sb[0:64, pr, gl], start=True, stop=True)
                        nc.tensor.matmul(
                            scAB[:, qw:2 * qw], KT_sb[64:128, pr, kcs],
                            QT_sb[64:128, pr, gl], start=True, stop=True)
                        pT = pTp.tile([128, 1024], bf16, tag="pT", name="pT")
                        nc.scalar.activation(pT[:], scAB[:], Exp,
                                             scale=float(SCALE / (WSC * WSC)))

                        def av_batch(kc=kc, pT=pT, avA=avA, avB=avB,
                                     hA=hA, hB=hB):
                            for hoff, h, av in ((0, hA, avA), (qw, hB, avB)):
                                for qc in range(4):
                                    nc.tensor.matmul(
                                        av[:, qc, :],
                                        pT[:, hoff + qc * 128:
                                           hoff + (qc + 1) * 128],
                                        V_sb[:, kc, h * 65:h * 65 + 65],
                                        start=(kc == 0 and qc == 0),
                                        stop=(kc == sch - 1 and qc == 3))
                        slot = u * 16 + kc
                        clk["vpe"] += 427.0
                        clk["vact"] = max(clk["vact"],
                                          clk["vpe"] + 100.0) + 1038.0
                        # PE can never really be more than ~2 slots behind
                        # ACT (ACT waits on scores); clamping kills phantom
                        # deficit that would otherwise trigger filler bursts
                        # that wall off the next scores in the PE queue.
                        clk["vpe"] = max(clk["vpe"], clk["vact"] - CFG["clamp"])
                        # AV(kc) also needs V_sb[:, kc] (vT DMA + V-proj
                        # filler); during the DMA-bound opening don't emit
                        # it before the data can be there, or the in-order
                        # PE queue stalls behind the wait.
                        av_es = max(slot + 1,
                                    aslot(f"v{(kc // 4) * 4 + 3}") + 1)
                        deferred.append((av_es, 217.0, av_batch))
                        emit_deferred(slot)
                        pull(slot, CFG["era_budget"] if u < 2 else
                             (2 if u < 12 else CFG["tail_budget"]))

                    state = {}

                    def make_norm(avA=avA, avB=avB, state=state,
                                  final=(u == len(units) - 1)):
                        def norm():
                            recA = nrm.tile([128, 4], f32, tag="recA",
                                            name="recA")
                            recB = nrm.tile([128, 4], f32, tag="recB",
                                            name="recB")
                            avn = nrm.tile([128, 4, 128], bf16, tag="avn",
                                           name="avn")
                            if final:
                                srcA, srcB = avA, avB
                            else:
                                srcA = nrm.tile([128, 4, 65], f32, tag="cpA",
                                                name="cpA", bufs=1)
                                srcB = nrm.tile([128, 4, 65], f32, tag="cpB",
                                                name="cpB", bufs=1)
                                nc.vector.tensor_copy(srcA[:], avA[:])
                                nc.vector.tensor_copy(srcB[:], avB[:])
                            nc.vector.reciprocal(recA[:], srcA[:, :, 64])
                            for qc in range(4):
                                nc.vector.tensor_scalar_mul(
                                    avn[:, qc, 0:64], srcA[:, qc, 0:64],
                                    recA[:, qc:qc + 1])
                            nc.vector.reciprocal(recB[:], srcB[:, :, 64])
                            for qc in range(4):
                                nc.vector.tensor_scalar_mul(
                                    avn[:, qc, 64:128], srcB[:, qc, 0:64],
                                    recB[:, qc:qc + 1])
                            state["avn"] = avn
                        return norm

                    def make_transp(pr=pr, t=t, state=state):
                        def transp():
                            finish_cur()
                            avn = state["avn"]
                            aot = ps.tile([128, 512], bf16, tag="px",
                                          name="aot")
                            for qc in range(4):
                                nc.tensor.matmul(
                                    aot[:, qc * 128:(qc + 1) * 128],
                                    avn[:, qc, :], ident_sb[:],
                                    is_transpose=True,
                                    start=(qc == 0), stop=(qc == 3))
                            nc.vector.tensor_copy(
                                AOT_sb[:, pr, t * qw:(t + 1) * qw], aot[:])
                            done_t[t] += 1
                            if done_t[t] == PAIRS:
                                for mo in range(d // 128):
                                    oq.append((mo, t))
                        return transp

                    # av_es may push this unit's late AV batches past the
                    # norm's slot; the norm must read COMPLETE accumulators,
                    # so force-flush every pending AV before appending it.
                    emit_deferred(10 ** 9)
                    last = u * 16 + 15
                    deferred.append((last + CFG.get("norm_off", 1), 0.0, make_norm()))
                    deferred.append((last + CFG.get("transp_off", 3), 220.0, make_transp()))
                    if u == 0:
                        clk["on"] = True
                        clk["vpe"] = clk["vact"] - 400.0

            # ---------------- tail ----------------
            # Deferred right now: av(u15,kc15), norm(u15), transp(u15) (and
            # possibly stragglers). Overlap the final block's output
            # projection with the exp/norm/transpose tail: open partial
            # psum groups (contraction chunks kk=0..2, whose AOT pairs are
            # already written) for mo 0-3 in the freed sc ring, close them
            # with kk=3 after the last transpose.
            TAIL_T = units[-1][1]
            tail_a = ps.tile([128, 1024], f32, tag="sc", name="tail_a")
            for j in range(2):
                seg = tail_a[:, j * 512:(j + 1) * 512]
                for kk in range(mch - 1):
                    nc.tensor.matmul(
                        seg, Wo_sb[:, kk, j * 128:(j + 1) * 128],
                        AOT_sb[:, kk, TAIL_T * 512:(TAIL_T + 1) * 512],
                        start=(kk == 0), stop=False)
            # av(kc15) before the tail_b partials (tail_b reuses kc15's sc
            # buffer, so its matmuls gate on the last exp anyway)
            es0, cost0, av_last = deferred.pop(0)
            av_last()
            tail_b = ps.tile([128, 1024], f32, tag="sc", name="tail_b")
            for j in range(2):
                seg = tail_b[:, j * 512:(j + 1) * 512]
                for kk in range(mch - 1):
                    nc.tensor.matmul(
                        seg, Wo_sb[:, kk, (j + 2) * 128:(j + 3) * 128],
                        AOT_sb[:, kk, TAIL_T * 512:(TAIL_T + 1) * 512],
                        start=(kk == 0), stop=False)
            # norm + transpose of the last unit
            for es, cost, fn in deferred:
                fn()
            deferred[:] = []
            # close the partial groups with the last pair's contraction
            for j, seg_t in ((0, tail_a), (1, tail_a),
                             (2, tail_b), (3, tail_b)):
                seg = seg_t[:, (j % 2) * 512:(j % 2 + 1) * 512]
                nc.tensor.matmul(
                    seg, Wo_sb[:, mch - 1, j * 128:(j + 1) * 128],
                    AOT_sb[:, mch - 1, TAIL_T * 512:(TAIL_T + 1) * 512],
                    start=False, stop=True)
                ot = outp.tile([128, 512], bf16, tag="ot", name="ot")
                if j % 2 == 0:
                    nc.scalar.activation(
                        ot[:], seg, mybir.ActivationFunctionType.Copy)
                else:
                    nc.vector.tensor_copy(ot[:], seg)
                nc.sync.dma_start(
                    outT[j * 128:(j + 1) * 128,
                         TAIL_T * 512:(TAIL_T + 1) * 512], ot[:])

            # ---------------- epilogue ----------------
            drain(10 ** 6)
            finish_cur()
            ntag = 0
            while oq:
                mo, n0 = oq.popleft()
                if n0 == TAIL_T and mo < 4:
                    continue  # handled by the partial-group tail
                for _ in outproj_chunk(
                        mo, n0, ("px", "sc")[ntag % 2],
                        on_act=(CFG["act_copies"] and ntag % 2 == 1)):
                    pass
                ntag += 1

    if not dbg:
        _strip_ldweights(nc)
        _split_multiwaits(nc, cap=1)
    return nc


def _get_nc():
    if "nc" not in _CACHE:
        _CACHE["nc"] = build_mha_nc()
    return _CACHE["nc"]


def _split8(x):
    f8 = ml_dtypes.float8_e4m3
    hi = x.astype(f8)
    lo = (x - hi.astype(np.float32)).astype(f8)
    return hi, lo


def _packw(w):
    """[1024, 512] -> [128, 4096]: W[c*128+p, m*128+j] -> out[p, (m,c,j)],
    giving 1024B-contiguous per (p, m) chunks for full-rate DMA."""
    return np.ascontiguousarray(
        w.reshape(8, 128, 4, 128).transpose(1, 2, 0, 3).reshape(128, 4096))


def make_in_maps(q, k, v, Wq, bq, Wk, bk, Wv, bv, Wo, bo, **_ignored):
    """Shard + lay out the full inputs for the 8 cores."""
    bf = ml_dtypes.bfloat16
    q = np.asarray(q, np.float32)
    k = np.asarray(k, np.float32)
    v = np.asarray(v, np.float32)
    Wq = np.asarray(Wq, np.float32)
    Wk = np.asarray(Wk, np.float32)
    Wv = np.asarray(Wv, np.float32)
    Wo = np.asarray(Wo, np.float32)
    bq = np.asarray(bq, np.float32)
    bk = np.asarray(bk, np.float32)
    eye = np.eye(128, dtype=bf)

    xplanes = {}
    for name, x in (("q", q), ("k", k), ("v", v)):
        for b in range(B):
            xplanes[(name, b)] = _split8(np.ascontiguousarray(x[b].T))

    in_maps = []
    for c in range(8):
        b, gi = divmod(c, 2)
        gs = slice(gi * G, (gi + 1) * G)
        qh, ql = xplanes[("q", b)]
        kh, kl = xplanes[("k", b)]
        vh, vl = xplanes[("v", b)]
        Wqh, Wql = _split8(np.ascontiguousarray(Wq[:, gs]) * WSC)
        Wkh, Wkl = _split8(np.ascontiguousarray(Wk[:, gs]) * WSC)
        Wvh, Wvl = _split8(np.ascontiguousarray(Wv[:, gs]) * WSC)
        in_maps.append({
            "qTh": qh, "qTl": ql,
            "kTh": kh, "kTl": kl,
            "vTh": vh, "vTl": vl,
            "Wqh": _packw(Wqh), "Wql": _packw(Wql),
            "Wkh": _packw(Wkh), "Wkl": _packw(Wkl),
            "Wvh": _packw(Wvh), "Wvl": _packw(Wvl),
            "Wo": np.ascontiguousarray(Wo[gs, :]).astype(bf),
            "bq": np.ascontiguousarray(
                (bq[gs] * WSC).reshape(G // 128, 128).T),
            "bk": np.ascontiguousarray(
                (bk[gs] * WSC).reshape(G // 128, 128).T),
            "ident": eye,
        })
    return in_maps


def run(in_maps, trace=False, trace_kwargs=None):
    from concourse.bass_utils import run_bass_kernel_spmd

    nc = _get_nc()
    kw = {}
    if trace:
        kw["trace"] = True
        kw.update(trace_kwargs or {})
    return run_bass_kernel_spmd(nc, in_maps, core_ids=list(range(8)), **kw)


def kernel(q, k, v, Wq, bq, Wk, bk, Wv, bv, Wo, bo, **_ignored):
    in_maps = make_in_maps(q, k, v, Wq, bq, Wk, bk, Wv, bv, Wo, bo)
    res = run(in_maps)
    bo = np.asarray(bo, np.float32)
    bv64 = np.asarray(bv, np.float64)
    Wo64 = np.asarray(Wo, np.float64)
    # bv rides through the softmax weighting unchanged: out += bv @ Wo
    bo_eff = (bo.astype(np.float64) + bv64 @ Wo64).astype(np.float32)
    out = np.empty((B, S, D), np.float32)
    for b in range(B):
        acc = (res.results[2 * b]["outT"].astype(np.float32)
               + res.results[2 * b + 1]["outT"].astype(np.float32))
        out[b] = acc.T + bo_eff[None, :]
    return out


# revision 36
# speedup vs baseline: 2.0460x; 1.0006x over previous
"""Multi-head attention (B=4, S=2048, D=1024, H=16) on 8 TRN2 NeuronCores, v3.

Sharding: core c <- (batch b = c // 2, head-group g = c % 2); head-group =
8 heads = 512 projection dims. Per core:

    QT = (q[b] @ Wq_g)^T   [512, S]  (bf16 at 32x scale, head-major)
    KT = (k[b] @ Wk_g)^T   [512, S]
    V  =  v[b] @ Wv_g      [S, 8*65] (64 data cols at 32x + 32.0 ones col)
    attention per head pair, q-tiles of 512:
        scoresT = K_h Q_h^T -> exp (ScalarE, scale SCALE/1024 folds the 32x
        W pre-scales) -> P^T bf16
        AV with P^T chunks [128k,128q] stationary and V [128k,65] moving;
        col 64 = 32*denominator. normalize via DVE reciprocal -> bf16
        PE transpose -> attn_out^T
    outT_partial = Wo_g^T @ attn_outT  [1024, S]

v3 over v2: the Q/K/V projections run as fp8e4 DoubleRow matmuls. Host
splits x and 32*W into (hi, lo) e4m3 planes (hi+lo recovers ~2^-8 relative
precision, better than bf16); each 512-col projection block is 12 DoubleRow
instructions (3 terms x 4 k-chunk-pairs, dropping the lo*lo term) at 0.5
cycles/col, i.e. 6/8 of the bf16 column cost. Scores/AV stay bf16 (their
contraction geometry gives fp8 no win at equal accuracy). The ScalarE exp
stream (256 tiles x ~1.04us) is the pacer; projections/output fill PE slack.

Host: out[b] = (outT_{b,0} + outT_{b,1})^T + bo + bv @ Wo.
"""

import numpy as np
import ml_dtypes

B, S, D, H = 4, 2048, 1024, 16
HD = 64
G = D // 2          # per-core head-group width = 512
NH = G // HD        # heads per core = 8
PAIRS = NH // 2
SCALE = 1.0 / np.sqrt(HD)
WSC = 32.0          # host pre-scale on Wq/Wk/Wv (power of 2, exact)

_CACHE = {}

# schedule knobs (swept offline with TimelineSim)
CFG = {"tail_budget": 8, "wm1_pos": "late", "act_copies": True,
       "transp_off": 7,
       "warmups": 11,
       "era_budget": 10,
       "pull_guard": 1100.0, "clamp": 2200.0}


def _split_multiwaits(nc, cap=1):
    """The walrus build in this container rejects instructions carrying more
    than `cap` sem waits (Tile's tail drain has 3). Move extra waits onto
    no-op instructions inserted just before, on the same engine."""
    import concourse.mybir as mybir

    n = 0
    for func in nc.m.functions:
        for blk in func.blocks:
            insts = list(blk.instructions)
            new_insts = []
            changed = False
            for inst in insts:
                si = inst.sync_info
                if si is not None and si.on_wait and len(si.on_wait) > cap:
                    waits = list(si.on_wait)
                    extra, keep = waits[:-cap], waits[-cap:]
                    for j, w in enumerate(extra):
                        nop = mybir.InstNoOp(
                            name=f"{inst.name}-wsplit{j}",
                            sync_info=mybir.SyncInfo(on_wait=[w], on_update=[]),
                            engine=inst.engine,
                            bass_nofuse=True,
                        )
                        new_insts.append(nop)
                        n += 1
                    inst.sync_info = mybir.SyncInfo(
                        on_wait=keep, on_update=list(si.on_update)
                    )
                    changed = True
                new_insts.append(inst)
            if changed:
                blk.instructions = new_insts
    return n


def _strip_ldweights(nc):
    """Drop Tile's separate Ldweights instructions; walrus handles
    self-loading matmuls. LDWs that carry sems become NoOps."""
    import concourse.mybir as mybir

    n = 0
    for func in nc.m.functions:
        for blk in func.blocks:
            insts = list(blk.instructions)
            new_insts = []
            changed = False
            for inst in insts:
                if type(inst).__name__ == "InstLdweights":
                    si = inst.sync_info
                    if si is not None and (si.on_wait or si.on_update):
                        new_insts.append(mybir.InstNoOp(
                            name=inst.name,
                            sync_info=mybir.SyncInfo(
                                on_wait=list(si.on_wait),
                                on_update=list(si.on_update)),
                            engine=inst.engine,
                            bass_nofuse=True,
                        ))
                    n += 1
                    changed = True
                    continue
                if type(inst).__name__ == "InstMatmult":
                    inst.ldweights = True
                new_insts.append(inst)
            if changed:
                blk.instructions = new_insts
    return n


def build_mha_nc(dbg=None):
    import concourse.bass as bass
    import concourse.mybir as mybir
    import concourse.tile as tile

    dt = mybir.dt
    f32 = dt.float32
    bf16 = dt.bfloat16
    fp8 = dt.float8e4
    Exp = mybir.ActivationFunctionType.Exp
    DR = mybir.MatmulPerfMode.DoubleRow

    s, d, g = S, D, G
    mch = g // 128        # head-pair chunks = 4
    kch = d // 128        # contraction chunks over D = 8
    sch = s // 128        # S chunks (k-chunks in attention) = 16
    qw = 512
    nqw = s // qw         # attention q-tiles per pair = 4
    sn = s // 512         # 512-wide blocks over S = 4

    nc = bass.Bass("TRN2", target_bir_lowering=False, debug=False)

    qTh = nc.declare_dram_parameter("qTh", [d, s], fp8, isOutput=False)
    qTl = nc.declare_dram_parameter("qTl", [d, s], fp8, isOutput=False)
    kTh = nc.declare_dram_parameter("kTh", [d, s], fp8, isOutput=False)
    kTl = nc.declare_dram_parameter("kTl", [d, s], fp8, isOutput=False)
    vTh = nc.declare_dram_parameter("vTh", [d, s], fp8, isOutput=False)
    vTl = nc.declare_dram_parameter("vTl", [d, s], fp8, isOutput=False)
    # W planes host-packed as [128, mch, kch, 128] (1024B contiguous per
    # (p, m) so DMA runs at full rate) and flattened to [128, 4096].
    Wqh = nc.declare_dram_parameter("Wqh", [128, d * g // 128], fp8,
                                    isOutput=False)
    Wql = nc.declare_dram_parameter("Wql", [128, d * g // 128], fp8,
                                    isOutput=False)
    Wkh = nc.declare_dram_parameter("Wkh", [128, d * g // 128], fp8,
                                    isOutput=False)
    Wkl = nc.declare_dram_parameter("Wkl", [128, d * g // 128], fp8,
                                    isOutput=False)
    Wvh = nc.declare_dram_parameter("Wvh", [128, d * g // 128], fp8,
                                    isOutput=False)
    Wvl = nc.declare_dram_parameter("Wvl", [128, d * g // 128], fp8,
                                    isOutput=False)
    Wo = nc.declare_dram_parameter("Wo", [g, d], bf16, isOutput=False)
    bq = nc.declare_dram_parameter("bq", [128, mch], f32, isOutput=False)
    bk = nc.declare_dram_parameter("bk", [128, mch], f32, isOutput=False)
    ident = nc.declare_dram_parameter("ident", [128, 128], bf16,
                                      isOutput=False)
    outT = nc.declare_dram_parameter("outT", [d, s], bf16, isOutput=True)

    with tile.TileContext(nc) as tc:
        with (
            tc.tile_pool(name="const", bufs=1) as const,
            tc.tile_pool(name="acts", bufs=1) as acts,
            tc.tile_pool(name="inT", bufs=1) as inT_pool,
            tc.tile_pool(name="pTp", bufs=2) as pTp,
            tc.tile_pool(name="nrm", bufs=2) as nrm,
            tc.tile_pool(name="outsb", bufs=5) as outp,
            tc.tile_pool(name="ps", bufs=2,
                         space=bass.MemorySpace.PSUM) as ps,
        ):
            # ---- constants ----
            Wqh_sb = const.tile([128, mch, kch, 128], fp8)
            Wql_sb = const.tile([128, mch, kch, 128], fp8)
            Wkh_sb = const.tile([128, mch, kch, 128], fp8)
            Wkl_sb = const.tile([128, mch, kch, 128], fp8)
            Wvh_sb = const.tile([128, mch, kch, 128], fp8)
            Wvl_sb = const.tile([128, mch, kch, 128], fp8)
            Wo_sb = const.tile([128, mch, d], bf16)
            bq_sb = const.tile([128, mch], f32)
            bk_sb = const.tile([128, mch], f32)
            ident_sb = const.tile([128, 128], bf16)
            ones_sb = const.tile([128, 512], bf16)
            nc.vector.memset(ones_sb[:], 1.0)

            # ---- resident activations ----
            QT_sb = acts.tile([128, mch, s], bf16)   # Q^T head-major, 32x
            KT_sb = acts.tile([128, mch, s], bf16)
            # V natural [S, nh*65]: 64 data cols (32x) + a 32.0 ones column
            # per head (65th moving col of AV = 32 * softmax denominator).
            V_sb = acts.tile([128, sch, NH * 65], bf16)
            AOT_sb = acts.tile([128, mch, s], bf16)  # attn_out^T (scale 1)
            nc.vector.memset(
                V_sb.rearrange("p s (h c) -> p s h c", c=65)[:, :, :, 64:65],
                float(WSC))

            # fp8 hi/lo input planes, [128, kch, s]
            vTh_t = inT_pool.tile([128, kch, s], fp8, name="vTh_t")
            vTl_t = inT_pool.tile([128, kch, s], fp8, name="vTl_t")
            kTh_t = inT_pool.tile([128, kch, s], fp8, name="kTh_t")
            kTl_t = inT_pool.tile([128, kch, s], fp8, name="kTl_t")
            qTh_t = inT_pool.tile([128, kch, s], fp8, name="qTh_t")
            qTl_t = inT_pool.tile([128, kch, s], fp8, name="qTl_t")

            # DMA arrival tracking: DMA_ENGINES serializes at ~360B/ns, so
            # arrival(ns) ~= cum_bytes/0.36 + sem-prop. Fillers become
            # pull-eligible only once their inputs have landed; pulling
            # earlier would wall the in-order PE queue behind a DMA wait.
            arr = {"bytes": 0.0}

            def _track(key, nbytes):
                arr["bytes"] += nbytes
                arr[key] = 2400.0 + arr["bytes"] / 360.0 + 900.0

            def dma_block(src, t, c0, c1, key="x"):
                # one instruction moves the column block of ALL 8 d-chunks
                nc.sync.dma_start(
                    t[:, :, c0:c1],
                    src.rearrange("(c p) n -> p c n", p=128)[:, :, c0:c1])
                _track(key, (c1 - c0) * d)

            def dma_w(src, t, m0, m1, key="x"):
                # W m-chunk block, host-packed: contiguous 1024B runs
                nc.sync.dma_start(
                    t[:, m0:m1],
                    src.rearrange("p (m c j) -> p m c j",
                                  m=mch, c=kch)[:, m0:m1])
                _track(key, (m1 - m0) * 128 * d)

            # DMA order = consumption order; the sync (HWDGE) queue issues
            # in emission order and DMA_ENGINES serializes at ~360B/ns, so
            # earliest-deadline-first. Unit 0 is DMA-paced end to end: its
            # gate set is kT (4MB) + vT (4MB) + qT tile0 (1MB) + Wk/Wq m0
            # (0.5MB) + Wv cols 0:256 (0.5MB) ~= 10MB ~= 28us.
            # Both bias chains gate the first scores; order so qTl and kTl
            # land back to back. All blocks >=512B elements (full DMA rate).
            dma_w(Wqh, Wqh_sb, 0, 1, key="x")
            dma_block(qTh, qTh_t, 0, 512, key="x")
            dma_w(Wkh, Wkh_sb, 0, 1, key="x")
            dma_block(kTh, kTh_t, 0, 512, key="x")
            dma_w(Wql, Wql_sb, 0, 1, key="wq0")
            dma_block(qTl, qTl_t, 0, 512, key="q0")
            nc.sync.dma_start(bq_sb[:], bq[:])
            _track("bq", 128 * 8)
            nc.sync.dma_start(bk_sb[:], bk[:])
            _track("bk", 128 * 8)
            dma_w(Wkl, Wkl_sb, 0, 1, key="wk0")
            dma_block(kTl, kTl_t, 0, 512, key="k3")
            # first exp ~11us; unit 0/1 stream kc in DMA pace from here
            dma_block(vTh, vTh_t, 0, 512, key="x")
            dma_block(vTl, vTl_t, 0, 512, key="v3")
            dma_w(Wvh, Wvh_sb, 0, 1, key="x")
            dma_w(Wvl, Wvl_sb, 0, 1, key="wv0")
            dma_block(kTh, kTh_t, 512, 1024, key="x")
            dma_block(kTl, kTl_t, 512, 1024, key="k7")
            dma_block(vTh, vTh_t, 512, 1024, key="x")
            dma_block(vTl, vTl_t, 512, 1024, key="v7")
            dma_block(kTh, kTh_t, 1024, 1536, key="x")
            dma_block(kTl, kTl_t, 1024, 1536, key="k11")
            dma_block(vTh, vTh_t, 1024, 1536, key="x")
            dma_block(vTl, vTl_t, 1024, 1536, key="v11")
            dma_block(kTh, kTh_t, 1536, 2048, key="x")
            dma_block(kTl, kTl_t, 1536, 2048, key="k15")
            dma_block(vTh, vTh_t, 1536, 2048, key="x")
            dma_block(vTl, vTl_t, 1536, 2048, key="v15")
            # units 1-3 need only their qT tiles
            dma_block(qTh, qTh_t, 512, 1024, key="x")
            dma_block(qTl, qTl_t, 512, 1024, key="q1")
            dma_block(qTh, qTh_t, 1024, 1536, key="x")
            dma_block(qTl, qTl_t, 1024, 1536, key="q2")
            dma_block(qTh, qTh_t, 1536, 2048, key="x")
            dma_block(qTl, qTl_t, 1536, 2048, key="q3")
            nc.sync.dma_start(ident_sb[:], ident[:])
            arr["bytes"] += 128 * 256
            # pair 1-3 weights (first needed at u4/u8/u9)
            dma_w(Wqh, Wqh_sb, 1, 2, key="x")
            dma_w(Wql, Wql_sb, 1, 2, key="wq1")
            dma_w(Wkh, Wkh_sb, 1, 2, key="x")
            dma_w(Wkl, Wkl_sb, 1, 2, key="wk1")
            dma_w(Wvh, Wvh_sb, 1, 2, key="x")
            dma_w(Wvl, Wvl_sb, 1, 2, key="wv1")
            dma_w(Wqh, Wqh_sb, 2, 3, key="x")
            dma_w(Wql, Wql_sb, 2, 3, key="wq2")
            dma_w(Wkh, Wkh_sb, 2, 3, key="x")
            dma_w(Wkl, Wkl_sb, 2, 3, key="wk2")
            dma_w(Wvh, Wvh_sb, 2, 3, key="x")
            dma_w(Wvl, Wvl_sb, 2, 3, key="wv2")
            dma_w(Wqh, Wqh_sb, 3, 4, key="x")
            dma_w(Wql, Wql_sb, 3, 4, key="wq3")
            dma_w(Wkh, Wkh_sb, 3, 4, key="x")
            dma_w(Wkl, Wkl_sb, 3, 4, key="wk3")
            dma_w(Wvh, Wvh_sb, 3, 4, key="x")
            dma_w(Wvl, Wvl_sb, 3, 4, key="wv3")
            nc.sync.dma_start(
                Wo_sb[:], Wo.rearrange("(c p) n -> p c n", p=128))

            # arrival(ns) -> earliest pull slot (first exp ~ T0)
            T0 = 12300.0

            def aslot(*keys):
                t = max(arr[k] for k in keys)
                return int(max(0.0, (t - T0) / 1038.0)) + 1

            # ---------------- emission helpers ----------------
            CPAIRS = [(0, 2), (2, 4), (4, 6), (6, 8)]

            def v_proj_chunk(sc, glo, ghi):
                """V projection for s-chunk sc, g block [glo, glo+128)."""
                gb = glo // 128
                vp = ps.tile([128, 512], f32, tag="px", name="vp")
                scs = slice(sc * 128, (sc + 1) * 128)
                # stationary = vT planes, moving = packed Wv m-chunk
                terms = ([(vTh_t, Wvh_sb, c) for c in CPAIRS]
                         + [(vTl_t, Wvh_sb, c) for c in CPAIRS]
                         + [(vTh_t, Wvl_sb, c) for c in CPAIRS])
                for i, (xt, Wt, (c0, c1)) in enumerate(terms):
                    nc.tensor.matmul(
                        vp[:, 0:128],
                        xt[:, c0:c1, scs],
                        Wt[:, gb, c0:c1, :],
                        start=(i == 0), stop=(i == len(terms) - 1),
                        perf_mode=DR,
                    )
                    yield 27.0
                # bv is folded into bo on the host.
                h0 = glo // HD
                nc.vector.tensor_copy(
                    V_sb[:, sc].rearrange(
                        "p (h c) -> p h c", c=65)[:, h0:h0 + 2, 0:64],
                    vp[:, 0:128].rearrange("p (h c) -> p h c", c=64))
                yield 55.0

            def qk_proj_chunk(Wh, Wl, xh, xl, b_sb, dst, m, n0, n1):
                """Q/K projection chunk: m-chunk m (pair), cols [n0, n1)."""
                w = n1 - n0
                pp = ps.tile([128, 512], f32, tag="px", name="pp")
                terms = ([(Wh, xh, c) for c in CPAIRS]
                         + [(Wl, xh, c) for c in CPAIRS]
                         + [(Wh, xl, c) for c in CPAIRS])
                for i, (Wt, xt, (c0, c1)) in enumerate(terms):
                    nc.tensor.matmul(
                        pp[:, 0:w],
                        Wt[:, m, c0:c1, :],
                        xt[:, c0:c1, n0:n1],
                        start=(i == 0), stop=(i == len(terms) - 1),
                        perf_mode=DR,
                    )
                    yield 0.21 * w
                nc.vector.tensor_scalar_add(
                    dst[:, m, n0:n1], pp[:, 0:w], b_sb[:, m:m + 1])
                yield 20.0

            def outproj_chunk(mo, n0, tag, on_act=False):
                """Output projection chunk outT[mo*128:, n0*512:]."""
                op = ps.tile([128, 1024] if tag == "sc" else [128, 512],
                             f32, tag=tag, name="op")
                for kk in range(mch):
                    nc.tensor.matmul(
                        op[:, 0:512],
                        Wo_sb[:, kk, mo * 128:(mo + 1) * 128],
                        AOT_sb[:, kk, n0 * 512:(n0 + 1) * 512],
                        start=(kk == 0), stop=(kk == mch - 1))
                    yield 213.0
                ot = outp.tile([128, 512], bf16, tag="ot", name="ot")
                if on_act:
                    nc.scalar.activation(
                        ot[:], op[:, 0:512],
                        mybir.ActivationFunctionType.Copy)
                else:
                    nc.vector.tensor_copy(ot[:], op[:, 0:512])
                nc.sync.dma_start(
                    outT[mo * 128:(mo + 1) * 128,
                         n0 * 512:(n0 + 1) * 512], ot[:])
                yield 20.0

            # ---------------- prologue ----------------
            # PE p-state warmup: dependency-free matmuls burn the 3us ramp
            # while the first input DMAs stream.
            for _ in range(CFG["warmups"]):
                wt = ps.tile([128, 512], f32, tag="px", name="wt")
                nc.tensor.matmul(wt[:], ones_sb[:, 0:128], ones_sb[:],
                                 start=True, stop=True)
            # Just enough to start attention: Q(pair 0, qtile 0) and
            # K(pair 0) cols 0:512. Everything else streams in as fillers.
            for _ in qk_proj_chunk(Wqh_sb, Wql_sb, qTh_t, qTl_t, bq_sb,
                                   QT_sb, 0, 0, 512):
                pass
            for _ in qk_proj_chunk(Wkh_sb, Wkl_sb, kTh_t, kTl_t, bk_sb,
                                   KT_sb, 0, 0, 512):
                pass

            # ---------------- filler schedule ----------------
            # (deadline, earliest, gen): deadline = last slot before which
            # the chunk must be fully EMITTED (consumer correctness);
            # earliest = slot at which its DMA inputs have landed (pulling
            # before that would wall the in-order PE queue on a DMA wait).
            KKEY = {0: "k3", 1: "k7", 2: "k11", 3: "k15"}
            units = [(0, 0), (0, 1), (0, 2), (0, 3),
                     (1, 0), (1, 1), (1, 2), (1, 3),
                     (2, 0), (3, 0), (2, 1), (3, 1),
                     (2, 2), (3, 2), (2, 3), (3, 3)]
            uidx = {pt: i for i, pt in enumerate(units)}
            first_u = {0: 0, 1: 4, 2: 8, 3: 9}
            fillers = []
            for gb, (dl0, step) in enumerate(((0, 1), (30, 2),
                                              (64, 3), (80, 3))):
                fu = first_u[gb] * 16
                for sc in range(sch):
                    # V g-block gb = heads of pair gb; needed at the AV of
                    # (first unit of that pair, kc=sc). The AV itself is
                    # arrival-gated (av_es), so the deadline may be pushed
                    # to the data arrival to avoid premature force-drains.
                    es = aslot(f"wv{gb}", f"v{(sc // 4) * 4 + 3}")
                    # lift the spread deadline toward data arrival, but never
                    # past the hard consumer slot: the unit-end AV flush
                    # fires regardless of av_es, and every AV's V chunk must
                    # be emitted (force-drained) before it.
                    hard = max(fu + sc - 1, 1)
                    dl = min(max(min(dl0 + step * sc, hard), 1, es), hard)
                    fillers.append((dl, es,
                                    v_proj_chunk(sc, gb * 128, gb * 128 + 128)))
            for p in range(4):
                for n0 in range(sn):
                    if p == 0 and n0 == 0:
                        continue  # prologue
                    fillers.append((first_u[p] * 16 + 4 * n0 - 2,
                                    aslot(f"wk{p}", KKEY[n0]),
                                    qk_proj_chunk(Wkh_sb, Wkl_sb, kTh_t,
                                                  kTl_t, bk_sb, KT_sb, p,
                                                  n0 * 512, (n0 + 1) * 512)))
            for pr in range(PAIRS):
                for t in range(nqw):
                    if (pr, t) == (0, 0):
                        continue
                    fillers.append((uidx[(pr, t)] * 16,
                                    aslot(f"wq{pr}", f"q{t}"),
                                    qk_proj_chunk(Wqh_sb, Wql_sb, qTh_t,
                                                  qTl_t, bq_sb, QT_sb, pr,
                                                  t * 512, (t + 1) * 512)))
            fillers.sort(key=lambda x: x[0])
            fq = list(fillers)
            from collections import deque
            oq = deque()

            # SINGLE-FLIGHT px discipline (see v2): at most one open chunk
            # generator may be suspended at a time.
            cur = {"gen": None, "dl": None}

            def finish_cur():
                if cur["gen"] is not None:
                    for c in cur["gen"]:
                        clk["vpe"] += c or 213.0
                    cur["gen"] = None

            def pull(slot, budget):
                spent = 0
                while True:
                    if clk["on"]:
                        if clk["vpe"] + 120.0 > clk["vact"] - CFG["pull_guard"]:
                            return
                    elif spent >= budget:
                        return
                    if cur["gen"] is None:
                        pick = None
                        for i, (dl, es, _gen) in enumerate(fq):
                            if dl > slot + 32:
                                break
                            if es <= slot:
                                pick = i
                                break
                        if pick is not None:
                            cur["dl"], _, cur["gen"] = fq.pop(pick)
                        elif oq:
                            mo, n0 = oq.popleft()
                            cur["gen"] = outproj_chunk(mo, n0, "px")
                            cur["dl"] = None
                        else:
                            return
                    try:
                        clk["vpe"] += next(cur["gen"]) or 213.0
                        spent += 1
                    except StopIteration:
                        cur["gen"] = None

            def drain(slot):
                while True:
                    if (cur["gen"] is not None and cur["dl"] is not None
                            and cur["dl"] <= slot):
                        finish_cur()
                        continue
                    if fq and fq[0][0] <= slot:
                        finish_cur()
                        cur["dl"], _, cur["gen"] = fq.pop(0)
                        finish_cur()
                        continue
                    break

            # ---------------- attention ----------------
            deferred = []   # (earliest_slot, pe_cost_ns, closure)
            clk = {"vpe": 0.0, "vact": 0.0, "on": False}

            def emit_deferred(slot):
                rest = []
                for es, cost, fn in deferred:
                    if es <= slot:
                        fn()
                        clk["vpe"] += cost
                    else:
                        rest.append((es, cost, fn))
                deferred[:] = rest

            done_t = [0] * nqw
            for u, (pr, t) in enumerate(units):
                if True:
                    hA, hB = 2 * pr, 2 * pr + 1
                    gl = slice(t * qw, (t + 1) * qw)
                    avA = ps.tile([128, 4, 65], f32, tag="av", name="avA")
                    avB = ps.tile([128, 4, 65], f32, tag="av", name="avB")
                    for kc in range(sch):
                        drain(u * 16 + kc)
                        kcs = slice(kc * 128, kc * 128 + 128)
                        scAB = ps.tile([128, 1024], f32, tag="sc",
                                       name="scAB")
                        nc.tensor.matmul(
                            scAB[:, 0:qw], KT_sb[0:64, pr, kcs],
                            QT_sb[0:64, pr, gl], start=True, stop=True)
                        nc.tensor.matmul(
                            scAB[:, qw:2 * qw], KT_sb[64:128, pr, kcs],
                            QT_sb[64:128, pr, gl], start=True, stop=True)
                        pT = pTp.tile([128, 1024], bf16, tag="pT", name="pT")
                        nc.scalar.activation(pT[:], scAB[:], Exp,
                                             scale=float(SCALE / (WSC * WSC)))

                        def av_batch(kc=kc, pT=pT, avA=avA, avB=avB,
                                     hA=hA, hB=hB):
                            for hoff, h, av in ((0, hA, avA), (qw, hB, avB)):
                                for qc in range(4):
                                    nc.tensor.matmul(
                                        av[:, qc, :],
                                        pT[:, hoff + qc * 128:
                                           hoff + (qc + 1) * 128],
                                        V_sb[:, kc, h * 65:h * 65 + 65],
                                        start=(kc == 0 and qc == 0),
                                        stop=(kc == sch - 1 and qc == 3))
                        slot = u * 16 + kc
                        clk["vpe"] += 427.0
                        clk["vact"] = max(clk["vact"],
                                          clk["vpe"] + 100.0) + 1038.0
                        # PE can never really be more than ~2 slots behind
                        # ACT (ACT waits on scores); clamping kills phantom
                        # deficit that would otherwise trigger filler bursts
                        # that wall off the next scores in the PE queue.
                        clk["vpe"] = max(clk["vpe"], clk["vact"] - CFG["clamp"])
                        # AV(kc) also needs V_sb[:, kc] (vT DMA + V-proj
                        # filler); during the DMA-bound opening don't emit
                        # it before the data can be there, or the in-order
                        # PE queue stalls behind the wait.
                        av_es = max(slot + 1,
                                    aslot(f"v{(kc // 4) * 4 + 3}") + 1)
                        deferred.append((av_es, 217.0, av_batch))
                        emit_deferred(slot)
                        pull(slot, CFG["era_budget"] if u < 2 else
                             (2 if u < 12 else CFG["tail_budget"]))

                    state = {}

                    def make_norm(avA=avA, avB=avB, state=state,
                                  final=(u == len(units) - 1)):
                        def norm():
                            recA = nrm.tile([128, 4], f32, tag="recA",
                                            name="recA")
                            recB = nrm.tile([128, 4], f32, tag="recB",
                                            name="recB")
                            avn = nrm.tile([128, 4, 128], bf16, tag="avn",
                                           name="avn")
                            if final:
                                srcA, srcB = avA, avB
                            else:
                                srcA = nrm.tile([128, 4, 65], f32, tag="cpA",
                                                name="cpA", bufs=1)
                                srcB = nrm.tile([128, 4, 65], f32, tag="cpB",
                                                name="cpB", bufs=1)
                                nc.vector.tensor_copy(srcA[:], avA[:])
                                nc.vector.tensor_copy(srcB[:], avB[:])
                            nc.vector.reciprocal(recA[:], srcA[:, :, 64])
                            for qc in range(4):
                                nc.vector.tensor_scalar_mul(
                                    avn[:, qc, 0:64], srcA[:, qc, 0:64],
                                    recA[:, qc:qc + 1])
                            nc.vector.reciprocal(recB[:], srcB[:, :, 64])
                            for qc in range(4):
                                nc.vector.tensor_scalar_mul(
                                    avn[:, qc, 64:128], srcB[:, qc, 0:64],
                                    recB[:, qc:qc + 1])
                            state["avn"] = avn
                        return norm

                    def make_transp(pr=pr, t=t, state=state):
                        def transp():
                            finish_cur()
                            avn = state["avn"]
                            aot = ps.tile([128, 512], bf16, tag="px",
                                          name="aot")
                            for qc in range(4):
                                nc.tensor.matmul(
                                    aot[:, qc * 128:(qc + 1) * 128],
                                    avn[:, qc, :], ident_sb[:],
                                    is_transpose=True,
                                    start=(qc == 0), stop=(qc == 3))
                            nc.vector.tensor_copy(
                                AOT_sb[:, pr, t * qw:(t + 1) * qw], aot[:])
                            done_t[t] += 1
                            if done_t[t] == PAIRS:
                                for mo in range(d // 128):
                                    oq.append((mo, t))
                        return transp

                    # av_es may push this unit's late AV batches past the
                    # norm's slot; the norm must read COMPLETE accumulators,
                    # so force-flush every pending AV before appending it.
                    emit_deferred(10 ** 9)
                    last = u * 16 + 15
                    deferred.append((last + CFG.get("norm_off", 1), 0.0, make_norm()))
                    deferred.append((last + CFG.get("transp_off", 3), 220.0, make_transp()))
                    if u == 0:
                        clk["on"] = True
                        clk["vpe"] = clk["vact"] - 400.0

            # ---------------- tail ----------------
            # Deferred right now: av(u15,kc15), norm(u15), transp(u15) (and
            # possibly stragglers). Overlap the final block's output
            # projection with the exp/norm/transpose tail: open partial
            # psum groups (contraction chunks kk=0..2, whose AOT pairs are
            # already written) for mo 0-3 in the freed sc ring, close them
            # with kk=3 after the last transpose.
            TAIL_T = units[-1][1]
            tail_a = ps.tile([128, 1024], f32, tag="sc", name="tail_a")
            for j in range(2):
                seg = tail_a[:, j * 512:(j + 1) * 512]
                for kk in range(mch - 1):
                    nc.tensor.matmul(
                        seg, Wo_sb[:, kk, j * 128:(j + 1) * 128],
                        AOT_sb[:, kk, TAIL_T * 512:(TAIL_T + 1) * 512],
                        start=(kk == 0), stop=False)
            # av(kc15) before the tail_b partials (tail_b reuses kc15's sc
            # buffer, so its matmuls gate on the last exp anyway)
            es0, cost0, av_last = deferred.pop(0)
            av_last()
            tail_b = ps.tile([128, 1024], f32, tag="sc", name="tail_b")
            for j in range(2):
                seg = tail_b[:, j * 512:(j + 1) * 512]
                for kk in range(mch - 1):
                    nc.tensor.matmul(
                        seg, Wo_sb[:, kk, (j + 2) * 128:(j + 3) * 128],
                        AOT_sb[:, kk, TAIL_T * 512:(TAIL_T + 1) * 512],
                        start=(kk == 0), stop=False)
            # norm + transpose of the last unit
            for es, cost, fn in deferred:
                fn()
            deferred[:] = []
            # close the partial groups with the last pair's contraction
            for j, seg_t in ((0, tail_a), (1, tail_a),
                             (2, tail_b), (3, tail_b)):
                seg = seg_t[:, (j % 2) * 512:(j % 2 + 1) * 512]
                nc.tensor.matmul(
                    seg, Wo_sb[:, mch - 1, j * 128:(j + 1) * 128],
                    AOT_sb[:, mch - 1, TAIL_T * 512:(TAIL_T + 1) * 512],
                    start=False, stop=True)
                ot = outp.tile([128, 512], bf16, tag="ot", name="ot")
                if j % 2 == 0:
                    nc.scalar.activation(
                        ot[:], seg, mybir.ActivationFunctionType.Copy)
                else:
                    nc.vector.tensor_copy(ot[:], seg)
                nc.sync.dma_start(
                    outT[j * 128:(j + 1) * 128,
                         TAIL_T * 512:(TAIL_T + 1) * 512], ot[:])

            # ---------------- epilogue ----------------
            drain(10 ** 6)
            finish_cur()
            ntag = 0
            while oq:
                mo, n0 = oq.popleft()
                if n0 == TAIL_T and mo < 4:
                    continue  # handled by the partial-group tail
                for _ in outproj_chunk(
                        mo, n0, ("px", "sc")[ntag % 2],
                        on_act=(CFG["act_copies"] and ntag % 2 == 1)):
                    pass
                ntag += 1

    if not dbg:
        _strip_ldweights(nc)
        _split_multiwaits(nc, cap=1)
    return nc


def _get_nc():
    if "nc" not in _CACHE:
        _CACHE["nc"] = build_mha_nc()
    return _CACHE["nc"]


def _split8(x):
    f8 = ml_dtypes.float8_e4m3
    hi = x.astype(f8)
    lo = (x - hi.astype(np.float32)).astype(f8)
    return hi, lo


def _packw(w):
    """[1024, 512] -> [128, 4096]: W[c*128+p, m*128+j] -> out[p, (m,c,j)],
    giving 1024B-contiguous per (p, m) chunks for full-rate DMA."""
    return np.ascontiguousarray(
        w.reshape(8, 128, 4, 128).transpose(1, 2, 0, 3).reshape(128, 4096))


def make_in_maps(q, k, v, Wq, bq, Wk, bk, Wv, bv, Wo, bo, **_ignored):
    """Shard + lay out the full inputs for the 8 cores."""
    bf = ml_dtypes.bfloat16
    q = np.asarray(q, np.float32)
    k = np.asarray(k, np.float32)
    v = np.asarray(v, np.float32)
    Wq = np.asarray(Wq, np.float32)
    Wk = np.asarray(Wk, np.float32)
    Wv = np.asarray(Wv, np.float32)
    Wo = np.asarray(Wo, np.float32)
    bq = np.asarray(bq, np.float32)
    bk = np.asarray(bk, np.float32)
    eye = np.eye(128, dtype=bf)

    xplanes = {}
    for name, x in (("q", q), ("k", k), ("v", v)):
        for b in range(B):
            xplanes[(name, b)] = _split8(np.ascontiguousarray(x[b].T))

    in_maps = []
    for c in range(8):
        b, gi = divmod(c, 2)
        gs = slice(gi * G, (gi + 1) * G)
        qh, ql = xplanes[("q", b)]
        kh, kl = xplanes[("k", b)]
        vh, vl = xplanes[("v", b)]
        Wqh, Wql = _split8(np.ascontiguousarray(Wq[:, gs]) * WSC)
        Wkh, Wkl = _split8(np.ascontiguousarray(Wk[:, gs]) * WSC)
        Wvh, Wvl = _split8(np.ascontiguousarray(Wv[:, gs]) * WSC)
        in_maps.append({
            "qTh": qh, "qTl": ql,
            "kTh": kh, "kTl": kl,
            "vTh": vh, "vTl": vl,
            "Wqh": _packw(Wqh), "Wql": _packw(Wql),
            "Wkh": _packw(Wkh), "Wkl": _packw(Wkl),
            "Wvh": _packw(Wvh), "Wvl": _packw(Wvl),
            "Wo": np.ascontiguousarray(Wo[gs, :]).astype(bf),
            "bq": np.ascontiguousarray(
                (bq[gs] * WSC).reshape(G // 128, 128).T),
            "bk": np.ascontiguousarray(
                (bk[gs] * WSC).reshape(G // 128, 128).T),
            "ident": eye,
        })
    return in_maps


def run(in_maps, trace=False, trace_kwargs=None):
    from concourse.bass_utils import run_bass_kernel_spmd

    nc = _get_nc()
    kw = {}
    if trace:
        kw["trace"] = True
        kw.update(trace_kwargs or {})
    return run_bass_kernel_spmd(nc, in_maps, core_ids=list(range(8)), **kw)


def kernel(q, k, v, Wq, bq, Wk, bk, Wv, bv, Wo, bo, **_ignored):
    in_maps = make_in_maps(q, k, v, Wq, bq, Wk, bk, Wv, bv, Wo, bo)
    res = run(in_maps)
    bo = np.asarray(bo, np.float32)
    bv64 = np.asarray(bv, np.float64)
    Wo64 = np.asarray(Wo, np.float64)
    # bv rides through the softmax weighting unchanged: out += bv @ Wo
    bo_eff = (bo.astype(np.float64) + bv64 @ Wo64).astype(np.float32)
    out = np.empty((B, S, D), np.float32)
    for b in range(B):
        acc = (res.results[2 * b]["outT"].astype(np.float32)
               + res.results[2 * b + 1]["outT"].astype(np.float32))
        out[b] = acc.T + bo_eff[None, :]
    return out
